# revision 1
# baseline (speedup 1.0000x reference)
"""Trainium2 Bass kernel for nn_LocalEnergy (protein local-energy GNN).

kernel(**inputs) takes FULL unsharded inputs (B=128), shards B across 8
NeuronCores (16 samples/core, pure data parallel), runs one Bass kernel
SPMD, gathers per-core [16] energies into the full [128] output.
"""

import sys
import types
import numpy as np
from contextlib import ExitStack


def ensure_axon_hooks():
    """The container's antenv is a stub without axon_hooks; inject it so
    run_bass_kernel_spmd(trace=True) can NTFF-profile."""
    if "antenv.axon_hooks" in sys.modules:
        return
    import antenv

    hooks = types.ModuleType("antenv.axon_hooks")
    hooks._h = None

    def set_axon_ntff_profile_hook(h):
        hooks._h = h

    def get_axon_ntff_profile_hook():
        return hooks._h

    hooks.set_axon_ntff_profile_hook = set_axon_ntff_profile_hook
    hooks.get_axon_ntff_profile_hook = get_axon_ntff_profile_hook
    sys.modules["antenv.axon_hooks"] = hooks
    antenv.axon_hooks = hooks
    try:
        from trn_agent_boot.trn_boot import _ntff_profile_via_ctypes

        hook = _ntff_profile_via_ctypes("/opt/axon/libaxon_pjrt.so")
        if hook is not None:
            set_axon_ntff_profile_hook(hook)
    except Exception:
        pass


ensure_axon_hooks()

import concourse.bass as bass  # noqa: E402
import concourse.tile as tile  # noqa: E402
from concourse import mybir, bacc, bass_utils  # noqa: E402

dt = mybir.dt
AF = mybir.ActivationFunctionType
ALU = mybir.AluOpType
AX = mybir.AxisListType

NCORES = 8
B, L, NAA, E, H = 128, 2048, 20, 16, 128
BPC = B // NCORES
W = 512

# Stack row layout (rhs rows for the W1 matmuls; matmul partition bases
# must be 32-aligned, so scalar features live in the base-64 block)
R_SH0, R_SH1, R_SH2, R_SH3 = 0, 16, 32, 48
R_SC = 64                      # 64-65: sin(phi), cos(phi)
R_LEN = 66
R_COS = 67

D_L, D_T, D_P = 1 + 2 * E, 1 + 3 * E, 2 + 4 * E


def _sel_sum3():
    S = np.zeros((96, 16), np.float32)
    for c in range(3):
        for s in range(16):
            S[32 * c + s, s] = 1.0
    return S


def _iota_sp():
    v = np.full((128, 1), 1e30, np.float32)
    for k in range(4):
        for a in range(NAA):
            v[32 * k + a, 0] = float(a)
    return v


def _ones_sp():
    v = np.zeros((128, NAA), np.float32)
    for k in range(4):
        v[32 * k, :] = 1.0
    return v


def build_nc(bpc=BPC, ll=L):
    assert bpc % 4 == 0 and bpc <= 16 and ll % W == 0
    nchunk = ll // W
    ngroup = bpc // 4

    nc = bacc.Bacc("TRN2", target_bir_lowering=False, debug=False)

    R_d = nc.dram_tensor("R", (bpc, ll, 3), dt.float32, kind="ExternalInput")
    seq_d = nc.dram_tensor("seq", (bpc, ll), dt.int32, kind="ExternalInput")
    emb_d = nc.dram_tensor("emb", (NAA, E), dt.float32, kind="ExternalInput")
    mlp_d = {}
    for pref, d_in in (("fl", D_L), ("ft", D_T), ("fp", D_P)):
        mlp_d[pref] = dict(
            W1=nc.dram_tensor(f"{pref}_W1", (d_in, H), dt.float32, kind="ExternalInput"),
            b1=nc.dram_tensor(f"{pref}_b1", (H,), dt.float32, kind="ExternalInput"),
            W2=nc.dram_tensor(f"{pref}_W2", (H, H), dt.float32, kind="ExternalInput"),
            b2=nc.dram_tensor(f"{pref}_b2", (H,), dt.float32, kind="ExternalInput"),
            W3=nc.dram_tensor(f"{pref}_W3", (H, 1), dt.float32, kind="ExternalInput"),
            b3=nc.dram_tensor(f"{pref}_b3", (1,), dt.float32, kind="ExternalInput"),
        )
    S48_d = nc.dram_tensor("S48", (96, 16), dt.float32, kind="ExternalInput")
    iota_d = nc.dram_tensor("iota_sp", (128, 1), dt.float32, kind="ExternalInput")
    ones_d = nc.dram_tensor("ones_sp", (128, NAA), dt.float32, kind="ExternalInput")
    out_d = nc.dram_tensor("out", (1, bpc), dt.float32, kind="ExternalOutput")

    nl, nt, np_ = ll - 1, ll - 2, ll - 3

    with tile.TileContext(nc) as tc, ExitStack() as ctx:
        consts = ctx.enter_context(tc.tile_pool(name="consts", bufs=1))
        S48 = consts.tile([96, 16], dt.float32)
        nc.sync.dma_start(out=S48, in_=S48_d.ap())
        iota_sp = consts.tile([128, 1], dt.float32)
        nc.sync.dma_start(out=iota_sp, in_=iota_d.ap())
        ones_sp = consts.tile([128, NAA], dt.float32)
        nc.sync.dma_start(out=ones_sp, in_=ones_d.ap())

        emb_sp_f = consts.tile([128, E], dt.float32)
        for k in range(4):
            nc.sync.dma_start(out=emb_sp_f[32 * k : 32 * k + NAA, :], in_=emb_d.ap())
        emb_sp = consts.tile([128, E], dt.float16)
        nc.vector.tensor_copy(out=emb_sp, in_=emb_sp_f)

        wl, wt, wp = mlp_d["fl"]["W1"], mlp_d["ft"]["W1"], mlp_d["fp"]["W1"]
        # one zero-padded lhsT per MLP covering stack rows 0..67:
        #   p: sh0-3 -> W1_p[2:66], sc(64-65) -> W1_p[0:2]
        #   t: sh0-2 -> W1_t[1:49], cos(67) -> W1_t[0]
        #   l: sh0-1 -> W1_l[1:33], len(66) -> W1_l[0]
        w1f = {}
        for pref, src in (("fp", wp), ("ft", wt), ("fl", wl)):
            w1f[pref] = consts.tile([68, H], dt.float32, name=f"w1f_{pref}")
            nc.vector.memset(w1f[pref], 0.0)
        nc.sync.dma_start(out=w1f["fp"][0:64, :], in_=wp.ap()[2:66, :])
        nc.sync.dma_start(out=w1f["fp"][R_SC : R_SC + 2, :], in_=wp.ap()[0:2, :])
        nc.sync.dma_start(out=w1f["ft"][0:48, :], in_=wt.ap()[1:49, :])
        nc.sync.dma_start(out=w1f["ft"][R_COS : R_COS + 1, :], in_=wt.ap()[0:1, :])
        nc.sync.dma_start(out=w1f["fl"][0:32, :], in_=wl.ap()[1:33, :])
        nc.sync.dma_start(out=w1f["fl"][R_LEN : R_LEN + 1, :], in_=wl.ap()[0:1, :])
        w1b = {}
        for pref in ("fp", "ft", "fl"):
            w1b[pref] = consts.tile([68, H], dt.float16, name=f"w1b_{pref}")
            nc.vector.tensor_copy(out=w1b[pref], in_=w1f[pref])
        # len feature is stored centered (len - LEN0); fold LEN0*W1_l[0] into b1_l
        w1l0c = consts.tile([H, 1], dt.float32, name="w1l0c")
        nc.sync.dma_start(out=w1l0c, in_=wl.ap()[0:1, :].rearrange("o h -> h o"))
        w2, b1c, b2c, w3c = {}, {}, {}, {}
        for pref in ("fl", "ft", "fp"):
            w2f = consts.tile([H, H], dt.float32, name=f"w2f_{pref}")
            nc.sync.dma_start(out=w2f, in_=mlp_d[pref]["W2"].ap())
            w2[pref] = consts.tile([H, H], dt.float16, name=f"w2_{pref}")
            nc.vector.tensor_copy(out=w2[pref], in_=w2f)
            b1c[pref] = consts.tile([H, 1], dt.float32, name=f"b1_{pref}")
            nc.sync.dma_start(out=b1c[pref], in_=mlp_d[pref]["b1"].ap().rearrange("(h o) -> h o", o=1))
            b2c[pref] = consts.tile([H, 1], dt.float32, name=f"b2_{pref}")
            nc.sync.dma_start(out=b2c[pref], in_=mlp_d[pref]["b2"].ap().rearrange("(h o) -> h o", o=1))
            w3c[pref] = consts.tile([H, 1], dt.float32, name=f"w3_{pref}")
            nc.sync.dma_start(out=w3c[pref], in_=mlp_d[pref]["W3"].ap())
        b1c["fl_adj"] = consts.tile([H, 1], dt.float32, name="b1_fl_adj")
        nc.vector.scalar_tensor_tensor(
            out=b1c["fl_adj"], in0=w1l0c, scalar=3.8, in1=b1c["fl"], op0=ALU.mult, op1=ALU.add
        )
        b3row = consts.tile([1, 3], dt.float32)
        for j, pref in enumerate(("fl", "ft", "fp")):
            nc.sync.dma_start(out=b3row[:, j : j + 1], in_=mlp_d[pref]["b3"].ap().rearrange("(o x) -> o x", o=1))

        seqf = consts.tile([bpc, ll], dt.float32, name="seqf")

        # feature rows (bf16) in natural [bpc, count] layout
        len_bf = consts.tile([16, nl], dt.float16, name="len_bf")
        cos_bf = consts.tile([16, nt], dt.float16, name="cos_bf")
        sin_p_bf = consts.tile([16, np_], dt.float16, name="sin_p_bf")
        cos_p_bf = consts.tile([16, np_], dt.float16, name="cos_p_bf")

        accl = consts.tile([H, bpc * nchunk], dt.float32, name="accl")
        acct = consts.tile([H, bpc * nchunk], dt.float32, name="acct")
        accp = consts.tile([H, bpc * nchunk], dt.float32, name="accp")

        # ================= Phase 1: geometry =================
        with tc.tile_pool(name="geo", bufs=1) as geo, \
             tc.tile_pool(name="geo_ps", bufs=2, space="PSUM") as geo_ps:
            seqi = geo.tile([bpc, ll], dt.int32, name="seqi", tag="seqi")
            nc.sync.dma_start(out=seqi, in_=seq_d.ap())
            nc.vector.tensor_copy(out=seqf, in_=seqi)

            rnat = geo.tile([bpc, 3 * ll], dt.float32, name="rnat", tag="rnat")
            nc.sync.dma_start(out=rnat, in_=R_d.ap().rearrange("b l c -> b (l c)"))
            rview = rnat.rearrange("b (l c) -> b l c", c=3)

            def g8(name):
                return geo.tile([96, nl], dt.float32, name=name, tag="g8", bufs=8)

            D = g8("D")
            for c in range(3):
                nc.vector.tensor_tensor(
                    out=D[32 * c : 32 * c + bpc, : nl],
                    in0=rview[:, 1:, c], in1=rview[:, : ll - 1, c], op=ALU.subtract,
                )
            Dr1 = g8("Dr1")
            Dr2 = g8("Dr2")
            for c in range(3):
                c1, c2 = (c + 1) % 3, (c + 2) % 3
                nc.gpsimd.tensor_copy(out=Dr1[32 * c : 32 * c + 16, :], in_=D[32 * c1 : 32 * c1 + 16, :])
                nc.gpsimd.tensor_copy(out=Dr2[32 * c : 32 * c + 16, :], in_=D[32 * c2 : 32 * c2 + 16, :])
            DSQ = g8("DSQ")
            nc.vector.tensor_tensor(out=DSQ, in0=D, in1=D, op=ALU.mult)
            DD = g8("DD")
            nc.vector.tensor_tensor(out=DD[:, :nt], in0=D[:, :nt], in1=D[:, 1:], op=ALU.mult)

            C = g8("C")
            t_a = g8("t_a")
            nc.vector.tensor_tensor(out=t_a[:, :nt], in0=Dr1[:, :nt], in1=Dr2[:, 1:], op=ALU.mult)
            nc.vector.tensor_tensor(out=C[:, :nt], in0=Dr2[:, :nt], in1=Dr1[:, 1:], op=ALU.mult)
            nc.vector.scalar_tensor_tensor(out=C[:, :nt], in0=C[:, :nt], scalar=-1.0, in1=t_a[:, :nt], op0=ALU.mult, op1=ALU.add)

            Cr1 = g8("Cr1")
            Cr2 = g8("Cr2")
            for c in range(3):
                c1, c2 = (c + 1) % 3, (c + 2) % 3
                nc.gpsimd.tensor_copy(out=Cr1[32 * c : 32 * c + 16, :nt], in_=C[32 * c1 : 32 * c1 + 16, :nt])
                nc.gpsimd.tensor_copy(out=Cr2[32 * c : 32 * c + 16, :nt], in_=C[32 * c2 : 32 * c2 + 16, :nt])

            M = g8("M")
            nc.vector.tensor_tensor(out=M[:, :np_], in0=Cr2[:, :np_], in1=Dr1[:, 1 : 1 + np_], op=ALU.mult)
            t_b = g8("t_b")
            nc.vector.tensor_tensor(out=t_b[:, :np_], in0=Cr1[:, :np_], in1=Dr2[:, 1 : 1 + np_], op=ALU.mult)
            nc.vector.scalar_tensor_tensor(out=M[:, :np_], in0=M[:, :np_], scalar=-1.0, in1=t_b[:, :np_], op0=ALU.mult, op1=ALU.add)

            XR = g8("XR")
            nc.vector.tensor_tensor(out=XR[:, :np_], in0=C[:, :np_], in1=C[:, 1 : 1 + np_], op=ALU.mult)
            YR = g8("YR")
            nc.vector.tensor_tensor(out=YR[:, :np_], in0=M[:, :np_], in1=C[:, 1 : 1 + np_], op=ALU.mult)

            def gps(name):
                return geo_ps.tile([16, nchunk, W], dt.float32, name=name, tag="gps")

            def selmm(dst, src, count):
                for c0 in range(0, count, W):
                    n = min(W, count - c0)
                    nc.tensor.matmul(dst[:, c0 // W, :n], S48, src[:, c0 : c0 + n], start=True, stop=True)

            rlen = geo.tile([16, nl], dt.float32, name="rlen", tag="rlen")
            lenf = geo.tile([16, nl], dt.float32, name="lenf", tag="lenf")

            lsq_ps = gps("lsq_ps")
            selmm(lsq_ps, DSQ, nl)
            for c0 in range(0, nl, W):
                n = min(W, nl - c0)
                nc.scalar.activation(out=rlen[:, c0 : c0 + n], in_=lsq_ps[:, c0 // W, :n], func=AF.Ln)
            nc.scalar.activation(out=rlen, in_=rlen, func=AF.Exp, scale=-0.5)
            for c0 in range(0, nl, W):
                n = min(W, nl - c0)
                nc.vector.tensor_tensor(out=lenf[:, c0 : c0 + n], in0=lsq_ps[:, c0 // W, :n], in1=rlen[:, c0 : c0 + n], op=ALU.mult)

            nc.gpsimd.tensor_scalar(out=len_bf, in0=lenf, scalar1=3.8, scalar2=None, op0=ALU.subtract)
            dot_ps = gps("dot_ps")
            selmm(dot_ps, DD, nt)
            tt1 = g8("tt1")
            for c0 in range(0, nt, W):
                n = min(W, nt - c0)
                nc.vector.tensor_tensor(out=tt1[:16, c0 : c0 + n], in0=dot_ps[:16, c0 // W, :n], in1=rlen[:, c0 : c0 + n], op=ALU.mult)
            nc.vector.scalar_tensor_tensor(out=tt1[:16, :nt], in0=tt1[:16, :nt], scalar=-1.0, in1=rlen[:, 1 : 1 + nt], op0=ALU.mult, op1=ALU.mult)
            nc.gpsimd.tensor_scalar(out=cos_bf, in0=tt1[:16, :nt], scalar1=-1.0, scalar2=1.0, op0=ALU.max, op1=ALU.min)

            xr_ps = gps("xr_ps")
            selmm(xr_ps, XR, np_)
            x_sb = g8("x_sb")
            for c0 in range(0, np_, W):
                n = min(W, np_ - c0)
                nc.scalar.activation(out=x_sb[:16, c0 : c0 + n], in_=xr_ps[:16, c0 // W, :n], func=AF.Copy)
            yr_ps = gps("yr_ps")
            selmm(yr_ps, YR, np_)
            y_sb = g8("y_sb")
            for c0 in range(0, np_, W):
                n = min(W, np_ - c0)
                nc.vector.tensor_tensor(out=y_sb[:16, c0 : c0 + n], in0=yr_ps[:16, c0 // W, :n], in1=rlen[:, 1 + c0 : 1 + c0 + n], op=ALU.mult)
            r2 = g8("r2")
            nc.vector.tensor_tensor(out=r2[:16, :np_], in0=x_sb[:16, :np_], in1=x_sb[:16, :np_], op=ALU.mult)
            t_c = g8("t_c")
            nc.vector.tensor_tensor(out=t_c[:16, :np_], in0=y_sb[:16, :np_], in1=y_sb[:16, :np_], op=ALU.mult)
            nc.vector.tensor_tensor(out=r2[:16, :np_], in0=r2[:16, :np_], in1=t_c[:16, :np_], op=ALU.add)
            nc.scalar.activation(out=r2[:16, :np_], in_=r2[:16, :np_], func=AF.Ln)
            nc.scalar.activation(out=r2[:16, :np_], in_=r2[:16, :np_], func=AF.Exp, scale=-0.5)
            nc.vector.tensor_tensor(out=sin_p_bf, in0=y_sb[:16, :np_], in1=r2[:16, :np_], op=ALU.mult)
            nc.vector.tensor_tensor(out=cos_p_bf, in0=x_sb[:16, :np_], in1=r2[:16, :np_], op=ALU.mult)

        # ================= Phase 2: embedding + MLPs =================
        with tc.tile_pool(name="grp", bufs=2) as grp, \
             tc.tile_pool(name="stk", bufs=3) as stk, \
             tc.tile_pool(name="mlp_sb", bufs=3) as mlp_sb, \
             tc.tile_pool(name="oh_ps", bufs=2, space="PSUM") as oh_ps, \
             tc.tile_pool(name="h1_ps", bufs=1, space="PSUM") as h1_ps, \
             tc.tile_pool(name="h2_ps", bufs=3, space="PSUM") as h2_ps:

            stack_bufs = []
            for i_ in range(3):
                sb_ = stk.tile([128, ll], dt.float16, name=f"stackbuf{i_}", bufs=1)
                nc.vector.memset(sb_, 0.0)
                stack_bufs.append(sb_)
            h1l_ps = h1_ps.tile([H, W], dt.float32, name="h1l")
            h1t_ps = h1_ps.tile([H, W], dt.float32, name="h1t")
            h1p_ps = h1_ps.tile([H, W], dt.float32, name="h1p")

            for g in range(ngroup):
                seqsp = grp.tile([128, ll], dt.float32, name="seqsp", tag="seqsp")
                for k in range(4):
                    nc.sync.dma_start(out=seqsp[32 * k : 32 * k + 1, :], in_=seqf[4 * g + k : 4 * g + k + 1, :])

                eT4 = grp.tile([128, ll], dt.float16, name="eT4", tag="eT4")
                for ci in range(nchunk):
                    c0 = ci * W
                    seqb = oh_ps.tile([128, W], dt.float32, name="seqb", tag="ohps")
                    for k in range(4):
                        nc.tensor.matmul(
                            seqb[32 * k : 32 * k + NAA, :],
                            ones_sp[32 * k : 32 * k + 1, :],
                            seqsp[32 * k : 32 * k + 1, c0 : c0 + W],
                            start=True, stop=True, tile_position=(32 * k, 32 * k),
                        )
                    oh4 = grp.tile([128, W], dt.float16, name="oh4", tag="oh4")
                    nc.vector.tensor_scalar(out=oh4, in0=seqb, scalar1=iota_sp, scalar2=None, op0=ALU.is_equal)
                    eTp = oh_ps.tile([128, W], dt.float32, name="eTp", tag="ohps")
                    for k in range(4):
                        nc.tensor.matmul(
                            eTp[32 * k : 32 * k + E, :],
                            emb_sp[32 * k : 32 * k + NAA, :],
                            oh4[32 * k : 32 * k + NAA, :],
                            start=True, stop=True, tile_position=(32 * k, 32 * k),
                        )
                    nc.scalar.activation(out=eT4[:, c0 : c0 + W], in_=eTp, func=AF.Copy)

                for k in range(4):
                    s = 4 * g + k
                    stack = stack_bufs[(4 * g + k) % 3]
                    for rr, woff in ((R_SH0, 0), (R_SH1, 1), (R_SH2, 2), (R_SH3, 3)):
                        nc.sync.dma_start(
                            out=stack[rr : rr + E, : ll - woff],
                            in_=eT4[32 * k : 32 * k + E, woff:ll],
                        )
                    nc.sync.dma_start(out=stack[R_LEN : R_LEN + 1, :nl], in_=len_bf[s : s + 1, :])
                    nc.sync.dma_start(out=stack[R_COS : R_COS + 1, :nt], in_=cos_bf[s : s + 1, :])
                    nc.sync.dma_start(out=stack[R_SC : R_SC + 1, :np_], in_=sin_p_bf[s : s + 1, :])
                    nc.sync.dma_start(out=stack[R_SC + 1 : R_SC + 2, :np_], in_=cos_p_bf[s : s + 1, :])

                    for ci in range(nchunk):
                        c0 = ci * W
                        last = ci == nchunk - 1
                        n_l = W - 1 if last else W
                        n_t = W - 2 if last else W
                        n_p = W - 3 if last else W
                        nc.tensor.matmul(
                            h1p_ps[:, :n_p], w1b["fp"], stack[0:68, c0 : c0 + n_p],
                            start=True, stop=True, tile_position=(0, 0),
                        )
                        nc.tensor.matmul(
                            h1t_ps[:, :n_t], w1b["ft"], stack[0:68, c0 : c0 + n_t],
                            start=True, stop=True, tile_position=(0, 0),
                        )
                        nc.tensor.matmul(
                            h1l_ps[:, :n_l], w1b["fl"], stack[0:68, c0 : c0 + n_l],
                            start=True, stop=True, tile_position=(0, 0),
                        )
                        col = s * nchunk + ci
                        for pref, h1ps, nn, eng in (
                            ("fl", h1l_ps, n_l, "vec"),
                            ("ft", h1t_ps, n_t, "vec"),
                            ("fp", h1p_ps, n_p, "vec"),
                        ):
                            h1r = mlp_sb.tile([H, W], dt.float16, name="h1r", tag="h1r")
                            if eng == "act":
                                nc.scalar.activation(out=h1r[:, :nn], in_=h1ps[:, :nn], func=AF.Relu, bias=b1c[pref])
                            else:
                                nc.vector.tensor_scalar(
                                    out=h1r[:, :nn], in0=h1ps[:, :nn],
                                    scalar1=b1c["fl_adj"] if pref == "fl" else b1c[pref],
                                    scalar2=0.0, op0=ALU.add, op1=ALU.max,
                                )
                            h2p = h2_ps.tile([H, W], dt.float32, name="h2p", tag="h2p")
                            nc.tensor.matmul(h2p[:, :nn], w2[pref], h1r[:, :nn], start=True, stop=True)
                            scr = mlp_sb.tile([H, W], dt.float16, name="scr", tag="scr")
                            acc = {"fl": accl, "ft": acct, "fp": accp}[pref]
                            if True:
                                nc.scalar.activation(
                                    out=scr[:, :nn], in_=h2p[:, :nn], func=AF.Relu,
                                    bias=b2c[pref], accum_out=acc[:, col : col + 1],
                                )
                            else:
                                nc.vector.tensor_scalar(
                                    out=scr[:, :nn], in0=h2p[:, :nn],
                                    scalar1=b2c[pref], scalar2=0.0, op0=ALU.add, op1=ALU.max,
                                    accum_out=acc[:, col : col + 1],
                                )

        # ================= final reduction =================
        with tc.tile_pool(name="fin_ps", bufs=1, space="PSUM") as fin_ps:
            ep = fin_ps.tile([1, 3, bpc * nchunk], dt.float32, name="ep")
            nc.tensor.matmul(ep[:, 0, :], w3c["fl"], accl, start=True, stop=True)
            nc.tensor.matmul(ep[:, 1, :], w3c["ft"], acct, start=True, stop=True)
            nc.tensor.matmul(ep[:, 2, :], w3c["fp"], accp, start=True, stop=True)
            esum = consts.tile([1, bpc], dt.float32, name="esum")
            nc.vector.tensor_reduce(
                out=esum,
                in_=ep.rearrange("o m (s c) -> o s m c", s=bpc),
                axis=AX.XY, op=ALU.add,
            )
            cnts = consts.tile([1, 3], dt.float32, name="cnts")
            nc.vector.memset(cnts[:, 0:1], float(nl))
            nc.vector.memset(cnts[:, 1:2], float(nt))
            nc.vector.memset(cnts[:, 2:3], float(np_))
            nc.vector.tensor_tensor(out=cnts, in0=cnts, in1=b3row, op=ALU.mult)
            b3sum = consts.tile([1, 1], dt.float32, name="b3sum")
            nc.vector.tensor_reduce(out=b3sum, in_=cnts, axis=AX.X, op=ALU.add)
            eout = consts.tile([1, bpc], dt.float32, name="eout")
            nc.vector.tensor_scalar(out=eout, in0=esum, scalar1=b3sum, scalar2=None, op0=ALU.add)
            nc.sync.dma_start(out=out_d.ap(), in_=eout)

    nc.finalize()
    return nc


_NC_CACHE = {}


def get_nc(bpc=BPC, ll=L):
    key = (bpc, ll)
    if key not in _NC_CACHE:
        _NC_CACHE[key] = build_nc(bpc, ll)
    return _NC_CACHE[key]


def make_in_maps(inputs, bpc=BPC, ncores=NCORES):
    consts = {"S48": _sel_sum3(), "iota_sp": _iota_sp(), "ones_sp": _ones_sp()}
    rep = {k: np.ascontiguousarray(np.asarray(v, np.float32)) for k, v in inputs.items()
           if k not in ("R", "seq")}
    R = np.ascontiguousarray(np.asarray(inputs["R"], np.float32))
    seq = np.ascontiguousarray(np.asarray(inputs["seq"], np.int32))
    in_maps = []
    for c in range(ncores):
        m = dict(consts)
        m.update(rep)
        m["R"] = R[c * bpc : (c + 1) * bpc]
        m["seq"] = seq[c * bpc : (c + 1) * bpc]
        in_maps.append(m)
    return in_maps


def kernel(**inputs):
    nc = get_nc()
    in_maps = make_in_maps(inputs)
    res = bass_utils.run_bass_kernel_spmd(nc, in_maps, core_ids=list(range(NCORES)))
    return np.concatenate([res.results[c]["out"][0] for c in range(NCORES)]).astype(np.float32)



# revision 8
# speedup vs baseline: 1.6888x; 1.6888x over previous
"""Trainium2 Bass kernel for nn_LocalEnergy (protein local-energy GNN).

kernel(**inputs) takes FULL unsharded inputs (B=128), shards B across 8
NeuronCores (16 samples/core, pure data parallel), runs one Bass kernel
SPMD, gathers per-core [16] energies into the full [128] output.

v2 layout:
 - Host prep (indexing/layout only): embedding gather emb[seq] replicated
   into 4 shifted row-blocks + ones row -> SE [16, 65, 2048] fp16 per core;
   R transposed to [3, 16, L]; W1 packed (zero-padded, bias folded via the
   ones row, torsion sin-row sign-flipped) to match the on-device stack
   row layout.
 - Device phase 1 (geometry): fp16 vector math, rotations/shifts via DMA
   (no gpsimd), fp16 selection matmuls, Ln/Exp on scalar engine. Produces
   feature tile F [64, L] fp16 = [sin | cos | len-3.8 | cos_theta] blocks.
 - Device phase 2 (MLPs): per sample, stack [69, L] fp16 = SE rows + F
   rows; 3x fused W1 matmuls (K=69) per 512-chunk, relu on vector engine
   per 1024-col half, W2 matmuls, and one scalar-engine Relu+accumulate
   over all 2047-ish columns per (sample, MLP).
"""

import sys
import types
import numpy as np
from contextlib import ExitStack


def ensure_axon_hooks():
    """The container's antenv is a stub without axon_hooks; inject it so
    run_bass_kernel_spmd(trace=True) can NTFF-profile."""
    if "antenv.axon_hooks" in sys.modules:
        return
    import antenv

    hooks = types.ModuleType("antenv.axon_hooks")
    hooks._h = None

    def set_axon_ntff_profile_hook(h):
        hooks._h = h

    def get_axon_ntff_profile_hook():
        return hooks._h

    hooks.set_axon_ntff_profile_hook = set_axon_ntff_profile_hook
    hooks.get_axon_ntff_profile_hook = get_axon_ntff_profile_hook
    sys.modules["antenv.axon_hooks"] = hooks
    antenv.axon_hooks = hooks
    try:
        from trn_agent_boot.trn_boot import _ntff_profile_via_ctypes

        hook = _ntff_profile_via_ctypes("/opt/axon/libaxon_pjrt.so")
        if hook is not None:
            set_axon_ntff_profile_hook(hook)
    except Exception:
        pass


ensure_axon_hooks()

import concourse.bass as bass  # noqa: E402
import concourse.tile as tile  # noqa: E402
from concourse import mybir, bacc, bass_utils  # noqa: E402

dt = mybir.dt
AF = mybir.ActivationFunctionType
ALU = mybir.AluOpType
AX = mybir.AxisListType

NCORES = 8
B, L, NAA, E, H = 128, 2048, 20, 16, 128
BPC = B // NCORES
W = 512
NCH = L // W                       # 4 chunks of 512 per sample
NL, NT, NP = L - 1, L - 2, L - 3
KROWS = 69                         # stack rows: 64 emb-shift + ones + 4 features
SINV = 1.0 / 16.0                  # cross-product scaling to stay in fp16 range

MLPS = ("fl", "ft", "fp")
KOFF = {"fl": 1, "ft": 2, "fp": 3}   # valid cols per sample = L - KOFF


def build_nc(bpc=BPC, ll=L):
    nc = bacc.Bacc("TRN2", target_bir_lowering=False, debug=False)

    Rt_d = nc.dram_tensor("Rt", (3, bpc, ll), dt.float32, kind="ExternalInput")
    SE_d = nc.dram_tensor("SE", (bpc, 65, ll), dt.float16, kind="ExternalInput")
    S48_d = nc.dram_tensor("S48", (96, 16), dt.float16, kind="ExternalInput")
    W1_d = nc.dram_tensor("W1P", (3, KROWS, H), dt.float16, kind="ExternalInput")
    W2_d = nc.dram_tensor("W2P", (3, H, H), dt.float16, kind="ExternalInput")
    B2_d = nc.dram_tensor("B2P", (3, H, 1), dt.float32, kind="ExternalInput")
    W3_d = nc.dram_tensor("W3P", (3, H, 1), dt.float32, kind="ExternalInput")
    B3_d = nc.dram_tensor("B3S", (1, 1), dt.float32, kind="ExternalInput")
    out_d = nc.dram_tensor("out", (1, bpc), dt.float32, kind="ExternalOutput")

    nl, nt, np_ = ll - 1, ll - 2, ll - 3

    with tile.TileContext(nc) as tc, ExitStack() as ctx:
        consts = ctx.enter_context(tc.tile_pool(name="consts", bufs=1))
        s48 = consts.tile([96, 16], dt.float16)
        nc.sync.dma_start(out=s48, in_=S48_d.ap())
        w1, w2, b2c, w3c = {}, {}, {}, {}
        for j, m in enumerate(MLPS):
            w1[m] = consts.tile([KROWS, H], dt.float16, name=f"w1_{m}")
            nc.sync.dma_start(out=w1[m], in_=W1_d.ap()[j])
            w2[m] = consts.tile([H, H], dt.float16, name=f"w2_{m}")
            nc.sync.dma_start(out=w2[m], in_=W2_d.ap()[j])
            b2c[m] = consts.tile([H, 1], dt.float32, name=f"b2_{m}")
            nc.sync.dma_start(out=b2c[m], in_=B2_d.ap()[j])
            w3c[m] = consts.tile([H, 1], dt.float32, name=f"w3_{m}")
            nc.sync.dma_start(out=w3c[m], in_=W3_d.ap()[j])
        b3s = consts.tile([1, 1], dt.float32, name="b3s")
        nc.sync.dma_start(out=b3s, in_=B3_d.ap())

        # feature tile: rows 32f+s, f = 0:sin' 1:cos(phi) 2:len-3.8 3:cos(theta)
        F = consts.tile([128, ll], dt.float16, name="F")
        nc.vector.memset(F, 0.0)
        eps_t = consts.tile([16, 1], dt.float32, name="eps_t")
        nc.vector.memset(eps_t, 1e-6)

        acc = {}
        for m in MLPS:
            acc[m] = consts.tile([H, bpc], dt.float32, name=f"acc_{m}")

        # stack pool opens before phase 1 so SE loads prefetch under geometry
        stk = ctx.enter_context(tc.tile_pool(name="stk", bufs=4))
        stack_tiles = {}

        def alloc_stack(s):
            t = stk.tile([KROWS, ll], dt.float16, name="stack", tag="stk")
            nc.sync.dma_start(out=t[0:65, :], in_=SE_d.ap()[s])
            stack_tiles[s] = t

        for s in range(4):
            alloc_stack(s)

        # ---------------- Phase 1: geometry ----------------
        with tc.tile_pool(name="geo", bufs=1) as geo, \
             tc.tile_pool(name="geo_ps", bufs=2, space="PSUM") as geo_ps:
            rt = geo.tile([96, ll], dt.float32, name="rt")
            nc.vector.memset(rt, 0.0)
            for c in range(3):
                nc.sync.dma_start(out=rt[32 * c: 32 * c + bpc, :], in_=Rt_d.ap()[c])

            def g16(name, cols):
                return geo.tile([96, cols], dt.float16, name=name)

            D = g16("D", nl)
            nc.vector.tensor_tensor(out=D, in0=rt[:, 1:ll], in1=rt[:, 0:nl], op=ALU.subtract)
            D1 = g16("D1", nt)
            nc.vector.tensor_tensor(out=D1, in0=rt[:, 2:ll], in1=rt[:, 1:nl], op=ALU.subtract)
            Ds = g16("Ds", nl)
            nc.vector.tensor_scalar(out=Ds, in0=D, scalar1=SINV, scalar2=None, op0=ALU.mult)
            D1s = g16("D1s", nt)
            nc.vector.tensor_scalar(out=D1s, in0=D1, scalar1=SINV, scalar2=None, op0=ALU.mult)

            DSQ = g16("DSQ", nl)
            nc.vector.tensor_tensor(out=DSQ, in0=D, in1=D, op=ALU.mult)
            DD = g16("DD", nt)
            nc.vector.tensor_tensor(out=DD, in0=D[:, 0:nt], in1=D1, op=ALU.mult)

            # coordinate rotations of the scaled bond vectors, via DMA
            A1 = g16("A1", nt)
            A2 = g16("A2", nt)
            B1 = g16("B1", nt)
            B2 = g16("B2", nt)
            for t in (A1, A2, B1, B2):
                nc.vector.memset(t, 0.0)
            for c in range(3):
                c1, c2 = (c + 1) % 3, (c + 2) % 3
                nc.sync.dma_start(out=A1[32 * c: 32 * c + bpc, :], in_=Ds[32 * c1: 32 * c1 + bpc, 0:nt])
                nc.sync.dma_start(out=A2[32 * c: 32 * c + bpc, :], in_=Ds[32 * c2: 32 * c2 + bpc, 0:nt])
                nc.sync.dma_start(out=B1[32 * c: 32 * c + bpc, :], in_=D1s[32 * c1: 32 * c1 + bpc, :])
                nc.sync.dma_start(out=B2[32 * c: 32 * c + bpc, :], in_=D1s[32 * c2: 32 * c2 + bpc, :])

            t_a = g16("t_a", nt)
            nc.vector.tensor_tensor(out=t_a, in0=A1, in1=B2, op=ALU.mult)
            t_b = g16("t_b", nt)
            nc.vector.tensor_tensor(out=t_b, in0=A2, in1=B1, op=ALU.mult)
            Cs = g16("Cs", nt)
            nc.vector.scalar_tensor_tensor(out=Cs, in0=t_b, scalar=-1.0, in1=t_a, op0=ALU.mult, op1=ALU.add)
            C1s = g16("C1s", np_)
            nc.vector.memset(C1s, 0.0)
            for c in range(3):
                nc.sync.dma_start(out=C1s[32 * c: 32 * c + bpc, :], in_=Cs[32 * c: 32 * c + bpc, 1: 1 + np_])

            XR = g16("XR", np_)
            nc.vector.tensor_tensor(out=XR, in0=Cs[:, 0:np_], in1=C1s, op=ALU.mult)
            YR = g16("YR", np_)
            nc.vector.tensor_tensor(out=YR, in0=Ds[:, 0:np_], in1=C1s, op=ALU.mult)

            def selmm(dst, src, count):
                for c0 in range(0, count, W):
                    n = min(W, count - c0)
                    nc.tensor.matmul(dst[:, c0 // W, :n], s48, src[:, c0: c0 + n], start=True, stop=True)

            lsq_ps = geo_ps.tile([16, NCH, W], dt.float32, name="lsq_ps", tag="gps")
            selmm(lsq_ps, DSQ, nl)
            lnl = geo.tile([16, nl], dt.float16, name="lnl")
            nc.scalar.activation(out=lnl, in_=lsq_ps.rearrange("p a b -> p (a b)")[:, 0:nl], func=AF.Ln)
            rlen = geo.tile([16, nl], dt.float16, name="rlen")
            nc.scalar.activation(out=rlen, in_=lnl, func=AF.Exp, scale=-0.5)
            lenf = geo.tile([16, nl], dt.float16, name="lenf")
            nc.scalar.activation(out=lenf, in_=lnl, func=AF.Exp, scale=0.5)
            # len feature: |d| - 3.8 (b1_fl adjusted host-side)
            nc.vector.tensor_scalar(out=F[64:80, 0:nl], in0=lenf, scalar1=3.8, scalar2=None, op0=ALU.subtract)

            rlen1 = geo.tile([16, nt], dt.float16, name="rlen1")
            nc.sync.dma_start(out=rlen1, in_=rlen[:, 1: 1 + nt])
            lenf1 = geo.tile([16, np_], dt.float16, name="lenf1")
            nc.sync.dma_start(out=lenf1, in_=lenf[:, 1: 1 + np_])

            dot_ps = geo_ps.tile([16, NCH, W], dt.float32, name="dot_ps", tag="gps")
            selmm(dot_ps, DD, nt)
            tt1 = geo.tile([16, nt], dt.float16, name="tt1")
            nc.vector.tensor_tensor(out=tt1, in0=dot_ps.rearrange("p a b -> p (a b)")[:, 0:nt], in1=rlen[:, 0:nt], op=ALU.mult)
            # cos(theta) = -(d.d') * rlen_i * rlen_{i+1}
            nc.vector.scalar_tensor_tensor(out=F[96:112, 0:nt], in0=tt1, scalar=-1.0, in1=rlen1, op0=ALU.mult, op1=ALU.mult)

            xr_ps = geo_ps.tile([16, NCH, W], dt.float32, name="xr_ps", tag="gps")
            selmm(xr_ps, XR, np_)
            x_sb = geo.tile([16, np_], dt.float16, name="x_sb")
            nc.scalar.activation(out=x_sb, in_=xr_ps.rearrange("p a b -> p (a b)")[:, 0:np_], func=AF.Copy)
            yr_ps = geo_ps.tile([16, NCH, W], dt.float32, name="yr_ps", tag="gps")
            selmm(yr_ps, YR, np_)
            y_sb = geo.tile([16, np_], dt.float16, name="y_sb")
            # y = (b1 . n2) * |b2|, extra 1/16 matches the s^4 scale of x
            nc.vector.scalar_tensor_tensor(
                out=y_sb, in0=yr_ps.rearrange("p a b -> p (a b)")[:, 0:np_],
                scalar=SINV, in1=lenf1, op0=ALU.mult, op1=ALU.mult)

            q1 = geo.tile([16, np_], dt.float16, name="q1")
            nc.vector.tensor_tensor(out=q1, in0=x_sb, in1=x_sb, op=ALU.mult)
            q2 = geo.tile([16, np_], dt.float16, name="q2")
            nc.vector.tensor_tensor(out=q2, in0=y_sb, in1=y_sb, op=ALU.mult)
            q = geo.tile([16, np_], dt.float16, name="q")
            nc.vector.tensor_tensor(out=q, in0=q1, in1=q2, op=ALU.add)
            lnq = geo.tile([16, np_], dt.float16, name="lnq")
            nc.scalar.activation(out=lnq, in_=q, func=AF.Ln, bias=eps_t)
            r2v = geo.tile([16, np_], dt.float16, name="r2v")
            nc.scalar.activation(out=r2v, in_=lnq, func=AF.Exp, scale=-0.5)
            # sin' = -sin(phi): sign folded into W1 sin rows host-side
            nc.vector.tensor_tensor(out=F[0:16, 0:np_], in0=y_sb, in1=r2v, op=ALU.mult)
            nc.vector.tensor_tensor(out=F[32:48, 0:np_], in0=x_sb, in1=r2v, op=ALU.mult)

        # ---------------- Phase 2: MLPs ----------------
        with tc.tile_pool(name="h1_ps", bufs=2, space="PSUM") as h1_ps, \
             tc.tile_pool(name="h1r_p", bufs=13) as h1r_p, \
             tc.tile_pool(name="h2_ps", bufs=1, space="PSUM") as h2_ps, \
             tc.tile_pool(name="scr_p", bufs=2) as scr_p:

            h1r_ref = {}

            def stage1(s):
                if s not in stack_tiles:
                    alloc_stack(s)
                if s + 2 < bpc and (s + 2) not in stack_tiles:
                    alloc_stack(s + 2)     # keep the SE prefetch 2 samples ahead
                stack = stack_tiles[s]
                # feature rows ride the scalar-engine HWDGE queue so they don't
                # head-block the SE prefetches on the sync queue
                for f in range(4):
                    nc.scalar.dma_start(out=stack[65 + f: 66 + f, :], in_=F[32 * f + s: 32 * f + s + 1, :])
                for m in MLPS:
                    for h in range(2):
                        h1 = h1_ps.tile([H, 2, W], dt.float32, name="h1", tag="h1ps")
                        for ci in range(2):
                            c0 = (2 * h + ci) * W
                            n = min(W, (ll - KOFF[m]) - c0)
                            nc.tensor.matmul(h1[:, ci, :n], w1[m], stack[:, c0: c0 + n], start=True, stop=True)
                        h1r = h1r_p.tile([H, 2, W], dt.float16, name="h1r", tag="h1r")
                        nc.vector.tensor_scalar(
                            out=h1r.rearrange("p a b -> p (a b)"),
                            in0=h1.rearrange("p a b -> p (a b)"),
                            scalar1=0.0, scalar2=None, op0=ALU.max)
                        h1r_ref[(s, m, h)] = h1r

            def stage2(s):
                for m in MLPS:
                    nv = ll - KOFF[m]
                    h2 = h2_ps.tile([H, NCH, W], dt.float32, name="h2", tag="h2ps")
                    for c in range(NCH):
                        c0 = c * W
                        n = min(W, nv - c0)
                        h1r = h1r_ref[(s, m, c // 2)]
                        nc.tensor.matmul(h2[:, c, :n], w2[m], h1r[:, c % 2, :n], start=True, stop=True)
                    scr = scr_p.tile([H, NCH, W], dt.float16, name="scr", tag="scr")
                    nc.scalar.activation(
                        out=scr.rearrange("p a b -> p (a b)")[:, 0:nv],
                        in_=h2.rearrange("p a b -> p (a b)")[:, 0:nv],
                        func=AF.Relu, bias=b2c[m],
                        accum_out=acc[m][:, s: s + 1])

            for s in range(bpc + 1):
                if s < bpc:
                    stage1(s)
                if s >= 1:
                    stage2(s - 1)

        # ---------------- final reduction ----------------
        with tc.tile_pool(name="fin_ps", bufs=1, space="PSUM") as fin_ps:
            ep = fin_ps.tile([1, 3, bpc], dt.float32, name="ep")
            for j, m in enumerate(MLPS):
                nc.tensor.matmul(ep[:, j, :], w3c[m], acc[m], start=True, stop=True)
            esum = consts.tile([1, bpc], dt.float32, name="esum")
            nc.vector.tensor_reduce(
                out=esum, in_=ep.rearrange("o m s -> o s m"), axis=AX.X, op=ALU.add)
            eout = consts.tile([1, bpc], dt.float32, name="eout")
            nc.vector.tensor_scalar(out=eout, in0=esum, scalar1=b3s, scalar2=None, op0=ALU.add)
            nc.sync.dma_start(out=out_d.ap(), in_=eout)

    nc.finalize()
    return nc


_NC_CACHE = {}


def get_nc(bpc=BPC, ll=L):
    key = (bpc, ll)
    if key not in _NC_CACHE:
        _NC_CACHE[key] = build_nc(bpc, ll)
    return _NC_CACHE[key]


def _sel48():
    S = np.zeros((96, 16), np.float16)
    for c in range(3):
        for s in range(16):
            S[32 * c + s, s] = 1.0
    return S


def pack_weights(inputs):
    """Pack per-MLP W1 into the [69, H] stack-row layout (fp16), fold b1 via
    the ones row, flip the torsion sin-row sign, fold the 3.8 len-centering
    into b1_fl."""
    f32 = lambda k: np.asarray(inputs[k], np.float32)
    W1P = np.zeros((3, KROWS, H), np.float32)
    # fl: x = [len, e0, e1]
    w = f32("fl_W1")
    W1P[0, 0:32] = w[1:33]
    W1P[0, 67] = w[0]
    W1P[0, 64] = f32("fl_b1") + 3.8 * w[0]
    # ft: x = [cos_t, e0, e1, e2]
    w = f32("ft_W1")
    W1P[1, 0:48] = w[1:49]
    W1P[1, 68] = w[0]
    W1P[1, 64] = f32("ft_b1")
    # fp: x = [sin, cos, e0, e1, e2, e3]
    w = f32("fp_W1")
    W1P[2, 0:64] = w[2:66]
    W1P[2, 65] = -w[0]          # device computes -sin
    W1P[2, 66] = w[1]
    W1P[2, 64] = f32("fp_b1")
    W2P = np.stack([f32(f"{m}_W2") for m in MLPS]).astype(np.float16)
    B2P = np.stack([f32(f"{m}_b2").reshape(H, 1) for m in MLPS]).astype(np.float32)
    W3P = np.stack([f32(f"{m}_W3") for m in MLPS]).astype(np.float32)
    b3sum = np.float32(
        float(np.asarray(inputs["fl_b3"]).reshape(-1)[0]) * NL
        + float(np.asarray(inputs["ft_b3"]).reshape(-1)[0]) * NT
        + float(np.asarray(inputs["fp_b3"]).reshape(-1)[0]) * NP
    )
    return W1P.astype(np.float16), W2P, B2P, W3P, np.array([[b3sum]], np.float32)


def make_in_maps(inputs, bpc=BPC, ncores=NCORES):
    W1P, W2P, B2P, W3P, B3S = pack_weights(inputs)
    emb16 = np.asarray(inputs["emb"], np.float32).astype(np.float16)
    seq = np.asarray(inputs["seq"], np.int64)
    R = np.asarray(inputs["R"], np.float32)
    e_all = emb16[seq]                       # [B, L, E]
    consts = dict(S48=_sel48(), W1P=W1P, W2P=W2P, B2P=B2P, W3P=W3P, B3S=B3S)
    in_maps = []
    for c in range(ncores):
        sl = slice(c * bpc, (c + 1) * bpc)
        Rt = np.ascontiguousarray(R[sl].transpose(2, 0, 1))        # [3, bpc, L]
        e = e_all[sl]                                              # [bpc, L, E]
        SE = np.zeros((bpc, 65, L), np.float16)
        for j in range(4):
            SE[:, 16 * j: 16 * j + 16, : L - j] = e[:, j:, :].transpose(0, 2, 1)
        SE[:, 64, :] = 1.0
        m = dict(consts)
        m["Rt"] = Rt
        m["SE"] = SE
        in_maps.append(m)
    return in_maps


def kernel(**inputs):
    nc = get_nc()
    in_maps = make_in_maps(inputs)
    res = bass_utils.run_bass_kernel_spmd(nc, in_maps, core_ids=list(range(NCORES)))
    return np.concatenate([res.results[c]["out"][0] for c in range(NCORES)]).astype(np.float32)


# revision 9
# speedup vs baseline: 1.8794x; 1.1128x over previous
"""Trainium2 Bass kernel for nn_LocalEnergy (protein local-energy GNN).

kernel(**inputs) takes FULL unsharded inputs (B=128), shards B across 8
NeuronCores (16 samples/core, pure data parallel), runs one Bass kernel
SPMD, gathers per-core [16] energies into the full [128] output.

v2 layout:
 - Host prep (indexing/layout only): embedding gather emb[seq] replicated
   into 4 shifted row-blocks + ones row -> SE [16, 65, 2048] fp16 per core;
   R transposed to [3, 16, L]; W1 packed (zero-padded, bias folded via the
   ones row, torsion sin-row sign-flipped) to match the on-device stack
   row layout.
 - Device phase 1 (geometry): fp16 vector math, rotations/shifts via DMA
   (no gpsimd), fp16 selection matmuls, Ln/Exp on scalar engine. Produces
   feature tile F [64, L] fp16 = [sin | cos | len-3.8 | cos_theta] blocks.
 - Device phase 2 (MLPs): per sample, stack [69, L] fp16 = SE rows + F
   rows; 3x fused W1 matmuls (K=69) per 512-chunk, relu on vector engine
   per 1024-col half, W2 matmuls, and one scalar-engine Relu+accumulate
   over all 2047-ish columns per (sample, MLP).
"""

import sys
import types
import numpy as np
from contextlib import ExitStack


def ensure_axon_hooks():
    """The container's antenv is a stub without axon_hooks; inject it so
    run_bass_kernel_spmd(trace=True) can NTFF-profile."""
    if "antenv.axon_hooks" in sys.modules:
        return
    import antenv

    hooks = types.ModuleType("antenv.axon_hooks")
    hooks._h = None

    def set_axon_ntff_profile_hook(h):
        hooks._h = h

    def get_axon_ntff_profile_hook():
        return hooks._h

    hooks.set_axon_ntff_profile_hook = set_axon_ntff_profile_hook
    hooks.get_axon_ntff_profile_hook = get_axon_ntff_profile_hook
    sys.modules["antenv.axon_hooks"] = hooks
    antenv.axon_hooks = hooks
    try:
        from trn_agent_boot.trn_boot import _ntff_profile_via_ctypes

        hook = _ntff_profile_via_ctypes("/opt/axon/libaxon_pjrt.so")
        if hook is not None:
            set_axon_ntff_profile_hook(hook)
    except Exception:
        pass


ensure_axon_hooks()

import concourse.bass as bass  # noqa: E402
import concourse.tile as tile  # noqa: E402
from concourse import mybir, bacc, bass_utils  # noqa: E402

dt = mybir.dt
AF = mybir.ActivationFunctionType
ALU = mybir.AluOpType
AX = mybir.AxisListType

NCORES = 8
B, L, NAA, E, H = 128, 2048, 20, 16, 128
BPC = B // NCORES
W = 512
NCH = L // W                       # 4 chunks of 512 per sample
NL, NT, NP = L - 1, L - 2, L - 3
KROWS = 69                         # stack rows: 64 emb-shift + ones + 4 features
SINV = 1.0 / 16.0                  # cross-product scaling to stay in fp16 range

MLPS = ("fl", "ft", "fp")
KOFF = {"fl": 1, "ft": 2, "fp": 3}   # valid cols per sample = L - KOFF


def build_nc(bpc=BPC, ll=L):
    nc = bacc.Bacc("TRN2", target_bir_lowering=False, debug=False)

    Rt_d = nc.dram_tensor("Rt", (3, bpc, ll), dt.float32, kind="ExternalInput")
    SE_d = nc.dram_tensor("SE", (bpc, 65, ll), dt.float16, kind="ExternalInput")
    S48_d = nc.dram_tensor("S48", (96, 16), dt.float16, kind="ExternalInput")
    W1_d = nc.dram_tensor("W1P", (3, KROWS, H), dt.float16, kind="ExternalInput")
    W2_d = nc.dram_tensor("W2P", (3, H, H), dt.float16, kind="ExternalInput")
    B2_d = nc.dram_tensor("B2P", (3, H, 1), dt.float32, kind="ExternalInput")
    W3_d = nc.dram_tensor("W3P", (3, H, 1), dt.float32, kind="ExternalInput")
    B3_d = nc.dram_tensor("B3S", (1, 1), dt.float32, kind="ExternalInput")
    out_d = nc.dram_tensor("out", (1, bpc), dt.float32, kind="ExternalOutput")

    nl, nt, np_ = ll - 1, ll - 2, ll - 3

    with tile.TileContext(nc) as tc, ExitStack() as ctx:
        consts = ctx.enter_context(tc.tile_pool(name="consts", bufs=1))
        s48 = consts.tile([96, 16], dt.float16)
        nc.sync.dma_start(out=s48, in_=S48_d.ap())
        w1, w2, b2c, w3c = {}, {}, {}, {}
        for j, m in enumerate(MLPS):
            w1[m] = consts.tile([KROWS, H], dt.float16, name=f"w1_{m}")
            nc.sync.dma_start(out=w1[m], in_=W1_d.ap()[j])
            w2[m] = consts.tile([H, H], dt.float16, name=f"w2_{m}")
            nc.sync.dma_start(out=w2[m], in_=W2_d.ap()[j])
            b2c[m] = consts.tile([H, 1], dt.float32, name=f"b2_{m}")
            nc.sync.dma_start(out=b2c[m], in_=B2_d.ap()[j])
            w3c[m] = consts.tile([H, 1], dt.float32, name=f"w3_{m}")
            nc.sync.dma_start(out=w3c[m], in_=W3_d.ap()[j])
        b3s = consts.tile([1, 1], dt.float32, name="b3s")
        nc.sync.dma_start(out=b3s, in_=B3_d.ap())

        # feature tile: rows 32f+s, f = 0:sin' 1:cos(phi) 2:len-3.8 3:cos(theta)
        F = consts.tile([128, ll], dt.float16, name="F")
        nc.vector.memset(F, 0.0)
        eps_t = consts.tile([16, 1], dt.float32, name="eps_t")
        nc.vector.memset(eps_t, 1e-6)

        acc = {}
        for m in MLPS:
            acc[m] = consts.tile([H, bpc], dt.float32, name=f"acc_{m}")

        # stack pool opens before phase 1 so SE loads prefetch under geometry
        stk = ctx.enter_context(tc.tile_pool(name="stk", bufs=4))
        stack_tiles = {}

        def alloc_stack(s):
            t = stk.tile([KROWS, ll], dt.float16, name="stack", tag="stk")
            nc.sync.dma_start(out=t[0:65, :], in_=SE_d.ap()[s])
            stack_tiles[s] = t

        for s in range(4):
            alloc_stack(s)

        # ---------------- Phase 1: geometry ----------------
        with tc.tile_pool(name="geo", bufs=1) as geo, \
             tc.tile_pool(name="geo_ps", bufs=2, space="PSUM") as geo_ps:
            rt = geo.tile([96, ll], dt.float32, name="rt")
            nc.vector.memset(rt, 0.0)
            for c in range(3):
                nc.sync.dma_start(out=rt[32 * c: 32 * c + bpc, :], in_=Rt_d.ap()[c])

            def g16(name, cols):
                return geo.tile([96, cols], dt.float16, name=name)

            # rotation/shift targets allocated+zeroed up front so the memsets
            # run during the initial DMA waits, off the DVE critical path
            A1 = g16("A1", nt)
            A2 = g16("A2", nt)
            B1 = g16("B1", nt)
            B2 = g16("B2", nt)
            C1s = g16("C1s", np_)
            for t in (A1, A2, B1, B2, C1s):
                nc.vector.memset(t, 0.0)

            D = g16("D", nl)
            nc.vector.tensor_tensor(out=D, in0=rt[:, 1:ll], in1=rt[:, 0:nl], op=ALU.subtract)
            D1 = g16("D1", nt)
            nc.vector.tensor_tensor(out=D1, in0=rt[:, 2:ll], in1=rt[:, 1:nl], op=ALU.subtract)
            Ds = g16("Ds", nl)
            nc.vector.tensor_scalar(out=Ds, in0=D, scalar1=SINV, scalar2=None, op0=ALU.mult)
            D1s = g16("D1s", nt)
            nc.vector.tensor_scalar(out=D1s, in0=D1, scalar1=SINV, scalar2=None, op0=ALU.mult)

            DSQ = g16("DSQ", nl)
            nc.vector.tensor_tensor(out=DSQ, in0=D, in1=D, op=ALU.mult)
            DD = g16("DD", nt)
            nc.vector.tensor_tensor(out=DD, in0=D[:, 0:nt], in1=D1, op=ALU.mult)

            # coordinate rotations of the scaled bond vectors, via DMA
            for c in range(3):
                c1, c2 = (c + 1) % 3, (c + 2) % 3
                nc.sync.dma_start(out=A1[32 * c: 32 * c + bpc, :], in_=Ds[32 * c1: 32 * c1 + bpc, 0:nt])
                nc.sync.dma_start(out=A2[32 * c: 32 * c + bpc, :], in_=Ds[32 * c2: 32 * c2 + bpc, 0:nt])
                nc.sync.dma_start(out=B1[32 * c: 32 * c + bpc, :], in_=D1s[32 * c1: 32 * c1 + bpc, :])
                nc.sync.dma_start(out=B2[32 * c: 32 * c + bpc, :], in_=D1s[32 * c2: 32 * c2 + bpc, :])

            t_a = g16("t_a", nt)
            nc.vector.tensor_tensor(out=t_a, in0=A1, in1=B2, op=ALU.mult)
            t_b = g16("t_b", nt)
            nc.vector.tensor_tensor(out=t_b, in0=A2, in1=B1, op=ALU.mult)
            Cs = g16("Cs", nt)
            nc.vector.scalar_tensor_tensor(out=Cs, in0=t_b, scalar=-1.0, in1=t_a, op0=ALU.mult, op1=ALU.add)
            for c in range(3):
                nc.sync.dma_start(out=C1s[32 * c: 32 * c + bpc, :], in_=Cs[32 * c: 32 * c + bpc, 1: 1 + np_])

            XR = g16("XR", np_)
            nc.vector.tensor_tensor(out=XR, in0=Cs[:, 0:np_], in1=C1s, op=ALU.mult)
            YR = g16("YR", np_)
            nc.vector.tensor_tensor(out=YR, in0=Ds[:, 0:np_], in1=C1s, op=ALU.mult)

            def selmm(dst, src, count):
                for c0 in range(0, count, W):
                    n = min(W, count - c0)
                    nc.tensor.matmul(dst[:, c0 // W, :n], s48, src[:, c0: c0 + n], start=True, stop=True)

            lsq_ps = geo_ps.tile([16, NCH, W], dt.float32, name="lsq_ps", tag="gps")
            selmm(lsq_ps, DSQ, nl)
            lnl = geo.tile([16, nl], dt.float16, name="lnl")
            nc.scalar.activation(out=lnl, in_=lsq_ps.rearrange("p a b -> p (a b)")[:, 0:nl], func=AF.Ln)
            rlen = geo.tile([16, nl], dt.float16, name="rlen")
            nc.scalar.activation(out=rlen, in_=lnl, func=AF.Exp, scale=-0.5)
            lenf = geo.tile([16, nl], dt.float16, name="lenf")
            nc.scalar.activation(out=lenf, in_=lnl, func=AF.Exp, scale=0.5)
            # len feature: |d| - 3.8 (b1_fl adjusted host-side)
            nc.vector.tensor_scalar(out=F[64:80, 0:nl], in0=lenf, scalar1=3.8, scalar2=None, op0=ALU.subtract)

            rlen1 = geo.tile([16, nt], dt.float16, name="rlen1")
            nc.sync.dma_start(out=rlen1, in_=rlen[:, 1: 1 + nt])
            lenf1 = geo.tile([16, np_], dt.float16, name="lenf1")
            nc.sync.dma_start(out=lenf1, in_=lenf[:, 1: 1 + np_])

            dot_ps = geo_ps.tile([16, NCH, W], dt.float32, name="dot_ps", tag="gps")
            selmm(dot_ps, DD, nt)
            tt1 = geo.tile([16, nt], dt.float16, name="tt1")
            nc.vector.tensor_tensor(out=tt1, in0=dot_ps.rearrange("p a b -> p (a b)")[:, 0:nt], in1=rlen[:, 0:nt], op=ALU.mult)
            # cos(theta) = -(d.d') * rlen_i * rlen_{i+1}
            nc.vector.scalar_tensor_tensor(out=F[96:112, 0:nt], in0=tt1, scalar=-1.0, in1=rlen1, op0=ALU.mult, op1=ALU.mult)

            xr_ps = geo_ps.tile([16, NCH, W], dt.float32, name="xr_ps", tag="gps")
            selmm(xr_ps, XR, np_)
            x_sb = geo.tile([16, np_], dt.float16, name="x_sb")
            nc.scalar.activation(out=x_sb, in_=xr_ps.rearrange("p a b -> p (a b)")[:, 0:np_], func=AF.Copy)
            yr_ps = geo_ps.tile([16, NCH, W], dt.float32, name="yr_ps", tag="gps")
            selmm(yr_ps, YR, np_)
            y_sb = geo.tile([16, np_], dt.float16, name="y_sb")
            # y = (b1 . n2) * |b2|, extra 1/16 matches the s^4 scale of x
            nc.vector.scalar_tensor_tensor(
                out=y_sb, in0=yr_ps.rearrange("p a b -> p (a b)")[:, 0:np_],
                scalar=SINV, in1=lenf1, op0=ALU.mult, op1=ALU.mult)

            q1 = geo.tile([16, np_], dt.float16, name="q1")
            nc.vector.tensor_tensor(out=q1, in0=x_sb, in1=x_sb, op=ALU.mult)
            q2 = geo.tile([16, np_], dt.float16, name="q2")
            nc.vector.tensor_tensor(out=q2, in0=y_sb, in1=y_sb, op=ALU.mult)
            q = geo.tile([16, np_], dt.float16, name="q")
            nc.vector.tensor_tensor(out=q, in0=q1, in1=q2, op=ALU.add)
            lnq = geo.tile([16, np_], dt.float16, name="lnq")
            nc.scalar.activation(out=lnq, in_=q, func=AF.Ln, bias=eps_t)
            r2v = geo.tile([16, np_], dt.float16, name="r2v")
            nc.scalar.activation(out=r2v, in_=lnq, func=AF.Exp, scale=-0.5)
            # sin' = -sin(phi): sign folded into W1 sin rows host-side
            nc.vector.tensor_tensor(out=F[0:16, 0:np_], in0=y_sb, in1=r2v, op=ALU.mult)
            nc.vector.tensor_tensor(out=F[32:48, 0:np_], in0=x_sb, in1=r2v, op=ALU.mult)

        # ---------------- Phase 2: MLPs ----------------
        with tc.tile_pool(name="h1_ps", bufs=2, space="PSUM") as h1_ps, \
             tc.tile_pool(name="h1r_p", bufs=13) as h1r_p, \
             tc.tile_pool(name="h2_ps", bufs=1, space="PSUM") as h2_ps, \
             tc.tile_pool(name="scr_p", bufs=2) as scr_p:

            h1r_ref = {}

            def emit_w1(s, m):
                stack = stack_tiles[s]
                for h in range(2):
                    h1 = h1_ps.tile([H, 2, W], dt.float32, name="h1", tag="h1ps")
                    for ci in range(2):
                        c0 = (2 * h + ci) * W
                        n = min(W, (ll - KOFF[m]) - c0)
                        nc.tensor.matmul(h1[:, ci, :n], w1[m], stack[:, c0: c0 + n], start=True, stop=True)
                    h1r = h1r_p.tile([H, 2, W], dt.float16, name="h1r", tag="h1r")
                    nc.vector.tensor_scalar(
                        out=h1r.rearrange("p a b -> p (a b)"),
                        in0=h1.rearrange("p a b -> p (a b)"),
                        scalar1=0.0, scalar2=None, op0=ALU.max)
                    h1r_ref[(s, m, h)] = h1r

            def emit_w2(s, m):
                nv = ll - KOFF[m]
                h2 = h2_ps.tile([H, NCH, W], dt.float32, name="h2", tag="h2ps")
                for c in range(NCH):
                    c0 = c * W
                    n = min(W, nv - c0)
                    h1r = h1r_ref.pop((s, m, c // 2)) if c % 2 else h1r_ref[(s, m, c // 2)]
                    nc.tensor.matmul(h2[:, c, :n], w2[m], h1r[:, c % 2, :n], start=True, stop=True)
                scr = scr_p.tile([H, NCH, W], dt.float16, name="scr", tag="scr")
                nc.scalar.activation(
                    out=scr.rearrange("p a b -> p (a b)")[:, 0:nv],
                    in_=h2.rearrange("p a b -> p (a b)")[:, 0:nv],
                    func=AF.Relu, bias=b2c[m],
                    accum_out=acc[m][:, s: s + 1])

            def prep_sample(s):
                if s not in stack_tiles:
                    alloc_stack(s)
                if s + 2 < bpc and (s + 2) not in stack_tiles:
                    alloc_stack(s + 2)     # keep the SE prefetch 2 samples ahead
                stack = stack_tiles[s]
                Fv = F.rearrange("(f s) l -> f s l", s=32)
                nc.sync.dma_start(out=stack[65:69, :], in_=Fv[:, s, :])

            for s in range(bpc + 1):
                if s < bpc:
                    prep_sample(s)
                for m in MLPS:
                    if s < bpc:
                        emit_w1(s, m)
                    if s >= 1:
                        emit_w2(s - 1, m)

        # ---------------- final reduction ----------------
        with tc.tile_pool(name="fin_ps", bufs=1, space="PSUM") as fin_ps:
            ep = fin_ps.tile([1, 3, bpc], dt.float32, name="ep")
            for j, m in enumerate(MLPS):
                nc.tensor.matmul(ep[:, j, :], w3c[m], acc[m], start=True, stop=True)
            esum = consts.tile([1, bpc], dt.float32, name="esum")
            nc.vector.tensor_reduce(
                out=esum, in_=ep.rearrange("o m s -> o s m"), axis=AX.X, op=ALU.add)
            eout = consts.tile([1, bpc], dt.float32, name="eout")
            nc.vector.tensor_scalar(out=eout, in0=esum, scalar1=b3s, scalar2=None, op0=ALU.add)
            nc.sync.dma_start(out=out_d.ap(), in_=eout)

    nc.finalize()
    return nc


_NC_CACHE = {}


def get_nc(bpc=BPC, ll=L):
    key = (bpc, ll)
    if key not in _NC_CACHE:
        _NC_CACHE[key] = build_nc(bpc, ll)
    return _NC_CACHE[key]


def _sel48():
    S = np.zeros((96, 16), np.float16)
    for c in range(3):
        for s in range(16):
            S[32 * c + s, s] = 1.0
    return S


def pack_weights(inputs):
    """Pack per-MLP W1 into the [69, H] stack-row layout (fp16), fold b1 via
    the ones row, flip the torsion sin-row sign, fold the 3.8 len-centering
    into b1_fl."""
    f32 = lambda k: np.asarray(inputs[k], np.float32)
    W1P = np.zeros((3, KROWS, H), np.float32)
    # fl: x = [len, e0, e1]
    w = f32("fl_W1")
    W1P[0, 0:32] = w[1:33]
    W1P[0, 67] = w[0]
    W1P[0, 64] = f32("fl_b1") + 3.8 * w[0]
    # ft: x = [cos_t, e0, e1, e2]
    w = f32("ft_W1")
    W1P[1, 0:48] = w[1:49]
    W1P[1, 68] = w[0]
    W1P[1, 64] = f32("ft_b1")
    # fp: x = [sin, cos, e0, e1, e2, e3]
    w = f32("fp_W1")
    W1P[2, 0:64] = w[2:66]
    W1P[2, 65] = -w[0]          # device computes -sin
    W1P[2, 66] = w[1]
    W1P[2, 64] = f32("fp_b1")
    W2P = np.stack([f32(f"{m}_W2") for m in MLPS]).astype(np.float16)
    B2P = np.stack([f32(f"{m}_b2").reshape(H, 1) for m in MLPS]).astype(np.float32)
    W3P = np.stack([f32(f"{m}_W3") for m in MLPS]).astype(np.float32)
    b3sum = np.float32(
        float(np.asarray(inputs["fl_b3"]).reshape(-1)[0]) * NL
        + float(np.asarray(inputs["ft_b3"]).reshape(-1)[0]) * NT
        + float(np.asarray(inputs["fp_b3"]).reshape(-1)[0]) * NP
    )
    return W1P.astype(np.float16), W2P, B2P, W3P, np.array([[b3sum]], np.float32)


def make_in_maps(inputs, bpc=BPC, ncores=NCORES):
    W1P, W2P, B2P, W3P, B3S = pack_weights(inputs)
    emb16 = np.asarray(inputs["emb"], np.float32).astype(np.float16)
    seq = np.asarray(inputs["seq"], np.int64)
    R = np.asarray(inputs["R"], np.float32)
    e_all = emb16[seq]                       # [B, L, E]
    consts = dict(S48=_sel48(), W1P=W1P, W2P=W2P, B2P=B2P, W3P=W3P, B3S=B3S)
    in_maps = []
    for c in range(ncores):
        sl = slice(c * bpc, (c + 1) * bpc)
        Rt = np.ascontiguousarray(R[sl].transpose(2, 0, 1))        # [3, bpc, L]
        e = e_all[sl]                                              # [bpc, L, E]
        SE = np.zeros((bpc, 65, L), np.float16)
        for j in range(4):
            SE[:, 16 * j: 16 * j + 16, : L - j] = e[:, j:, :].transpose(0, 2, 1)
        SE[:, 64, :] = 1.0
        m = dict(consts)
        m["Rt"] = Rt
        m["SE"] = SE
        in_maps.append(m)
    return in_maps


def kernel(**inputs):
    nc = get_nc()
    in_maps = make_in_maps(inputs)
    res = bass_utils.run_bass_kernel_spmd(nc, in_maps, core_ids=list(range(NCORES)))
    return np.concatenate([res.results[c]["out"][0] for c in range(NCORES)]).astype(np.float32)


# revision 12
# speedup vs baseline: 2.5257x; 1.3439x over previous
"""Trainium2 Bass kernel for nn_LocalEnergy (protein local-energy GNN).

kernel(**inputs) takes FULL unsharded inputs (B=128), shards B across 8
NeuronCores (16 samples/core, pure data parallel), runs one Bass kernel
SPMD, gathers per-core [16] energies into the full [128] output.

v2 layout:
 - Host prep (indexing/layout only): embedding gather emb[seq] replicated
   into 4 shifted row-blocks + ones row -> SE [16, 65, 2048] fp16 per core;
   R transposed to [3, 16, L]; W1 packed (zero-padded, bias folded via the
   ones row, torsion sin-row sign-flipped) to match the on-device stack
   row layout.
 - Device phase 1 (geometry): fp16 vector math, rotations/shifts via DMA
   (no gpsimd), fp16 selection matmuls, Ln/Exp on scalar engine. Produces
   feature tile F [64, L] fp16 = [sin | cos | len-3.8 | cos_theta] blocks.
 - Device phase 2 (MLPs): per sample, stack [69, L] fp16 = SE rows + F
   rows; 3x fused W1 matmuls (K=69) per 512-chunk, relu on vector engine
   per 1024-col half, W2 matmuls, and one scalar-engine Relu+accumulate
   over all 2047-ish columns per (sample, MLP).
"""

import sys
import types
import numpy as np
from contextlib import ExitStack


def ensure_axon_hooks():
    """The container's antenv is a stub without axon_hooks; inject it so
    run_bass_kernel_spmd(trace=True) can NTFF-profile."""
    if "antenv.axon_hooks" in sys.modules:
        return
    import antenv

    hooks = types.ModuleType("antenv.axon_hooks")
    hooks._h = None

    def set_axon_ntff_profile_hook(h):
        hooks._h = h

    def get_axon_ntff_profile_hook():
        return hooks._h

    hooks.set_axon_ntff_profile_hook = set_axon_ntff_profile_hook
    hooks.get_axon_ntff_profile_hook = get_axon_ntff_profile_hook
    sys.modules["antenv.axon_hooks"] = hooks
    antenv.axon_hooks = hooks
    try:
        from trn_agent_boot.trn_boot import _ntff_profile_via_ctypes

        hook = _ntff_profile_via_ctypes("/opt/axon/libaxon_pjrt.so")
        if hook is not None:
            set_axon_ntff_profile_hook(hook)
    except Exception:
        pass


ensure_axon_hooks()

import concourse.bass as bass  # noqa: E402
import concourse.tile as tile  # noqa: E402
from concourse import mybir, bacc, bass_utils  # noqa: E402

dt = mybir.dt
AF = mybir.ActivationFunctionType
ALU = mybir.AluOpType
AX = mybir.AxisListType

NCORES = 8
B, L, NAA, E, H = 128, 2048, 20, 16, 128
BPC = B // NCORES
W = 512
NCH = L // W                       # 4 chunks of 512 per sample
NL, NT, NP = L - 1, L - 2, L - 3
KROWS = 69                         # stack rows: 64 emb-shift + ones + 4 features
SINV = 1.0 / 16.0                  # cross-product scaling to stay in fp16 range

MLPS = ("fl", "ft", "fp")
KOFF = {"fl": 1, "ft": 2, "fp": 3}   # valid cols per sample = L - KOFF


def build_nc(bpc=BPC, ll=L):
    nc = bacc.Bacc("TRN2", target_bir_lowering=False, debug=False)

    Rt_d = nc.dram_tensor("Rt", (3, bpc, ll), dt.float32, kind="ExternalInput")
    SE_d = nc.dram_tensor("SE", (bpc, 65, ll), dt.float16, kind="ExternalInput")
    S48_d = nc.dram_tensor("S48", (96, 16), dt.float16, kind="ExternalInput")
    WW_d = nc.dram_tensor("WALL", (H, 6 * H), dt.float16, kind="ExternalInput")
    BW_d = nc.dram_tensor("BW", (H, 8), dt.float32, kind="ExternalInput")
    out_d = nc.dram_tensor("out", (1, bpc), dt.float32, kind="ExternalOutput")

    nl, nt, np_ = ll - 1, ll - 2, ll - 3

    with tile.TileContext(nc) as tc, ExitStack() as ctx:
        consts = ctx.enter_context(tc.tile_pool(name="consts", bufs=1))
        # rt first: phase 1 is gated on it, so its DMAs lead the sync queue
        rt = consts.tile([96, ll], dt.float32, name="rt")
        nc.vector.memset(rt, 0.0)
        for c in range(3):
            nc.sync.dma_start(out=rt[32 * c: 32 * c + bpc, :], in_=Rt_d.ap()[c])
        s48 = consts.tile([96, 16], dt.float16)
        nc.sync.dma_start(out=s48, in_=S48_d.ap())
        wall = consts.tile([H, 6 * H], dt.float16, name="wall")
        nc.sync.dma_start(out=wall, in_=WW_d.ap())
        bw = consts.tile([H, 8], dt.float32, name="bw")
        nc.sync.dma_start(out=bw, in_=BW_d.ap())
        w1, w2, b2c, w3c = {}, {}, {}, {}
        for j, m in enumerate(MLPS):
            w1[m] = wall[0:KROWS, H * j: H * (j + 1)]
            w2[m] = wall[:, H * (3 + j): H * (4 + j)]
            b2c[m] = bw[:, j: j + 1]
            w3c[m] = bw[:, 3 + j: 4 + j]
        b3s = bw[0:1, 6:7]

        # feature tile: rows 32f+s, f = 0:sin' 1:cos(phi) 2:len-3.8 3:cos(theta)
        F = consts.tile([128, ll], dt.float16, name="F")
        nc.vector.memset(F, 0.0)
        eps_t = consts.tile([16, 1], dt.float32, name="eps_t")
        nc.vector.memset(eps_t, 1e-6)

        acc = {}
        for m in MLPS:
            acc[m] = consts.tile([H, 2 * bpc], dt.float32, name=f"acc_{m}")

        # stack pool opens before phase 1 so SE loads prefetch under geometry
        stk = ctx.enter_context(tc.tile_pool(name="stk", bufs=4))
        stack_tiles = {}

        def alloc_stack(s):
            t = stk.tile([KROWS, ll], dt.float16, name="stack", tag="stk")
            nc.sync.dma_start(out=t[0:65, :], in_=SE_d.ap()[s])
            stack_tiles[s] = t

        for s in range(4):
            alloc_stack(s)

        # ---------------- Phase 1: geometry ----------------
        with tc.tile_pool(name="geo", bufs=1) as geo, \
             tc.tile_pool(name="geo_ps", bufs=2, space="PSUM") as geo_ps:
            def g16(name, cols):
                return geo.tile([96, cols], dt.float16, name=name)

            # rotation/shift targets allocated+zeroed up front so the memsets
            # run during the initial DMA waits, off the DVE critical path
            A1 = g16("A1", nt)
            A2 = g16("A2", nt)
            B1 = g16("B1", nt)
            B2 = g16("B2", nt)
            C1s = g16("C1s", np_)
            for t in (A1, A2, B1, B2, C1s):
                nc.vector.memset(t, 0.0)

            D = g16("D", nl)
            nc.vector.tensor_tensor(out=D, in0=rt[:, 1:ll], in1=rt[:, 0:nl], op=ALU.subtract)
            D1 = g16("D1", nt)
            nc.vector.tensor_tensor(out=D1, in0=rt[:, 2:ll], in1=rt[:, 1:nl], op=ALU.subtract)
            Ds = g16("Ds", nl)
            nc.vector.tensor_scalar(out=Ds, in0=D, scalar1=SINV, scalar2=None, op0=ALU.mult)
            D1s = g16("D1s", nt)
            nc.vector.tensor_scalar(out=D1s, in0=D1, scalar1=SINV, scalar2=None, op0=ALU.mult)

            DSQ = g16("DSQ", nl)
            nc.vector.tensor_tensor(out=DSQ, in0=D, in1=D, op=ALU.mult)
            DD = g16("DD", nt)
            nc.vector.tensor_tensor(out=DD, in0=D[:, 0:nt], in1=D1, op=ALU.mult)

            # coordinate rotations of the scaled bond vectors, via DMA
            for c in range(3):
                c1, c2 = (c + 1) % 3, (c + 2) % 3
                nc.sync.dma_start(out=A1[32 * c: 32 * c + bpc, :], in_=Ds[32 * c1: 32 * c1 + bpc, 0:nt])
                nc.sync.dma_start(out=A2[32 * c: 32 * c + bpc, :], in_=Ds[32 * c2: 32 * c2 + bpc, 0:nt])
                nc.sync.dma_start(out=B1[32 * c: 32 * c + bpc, :], in_=D1s[32 * c1: 32 * c1 + bpc, :])
                nc.sync.dma_start(out=B2[32 * c: 32 * c + bpc, :], in_=D1s[32 * c2: 32 * c2 + bpc, :])

            t_a = g16("t_a", nt)
            nc.vector.tensor_tensor(out=t_a, in0=A1, in1=B2, op=ALU.mult)
            t_b = g16("t_b", nt)
            nc.vector.tensor_tensor(out=t_b, in0=A2, in1=B1, op=ALU.mult)
            Cs = g16("Cs", nt)
            nc.vector.scalar_tensor_tensor(out=Cs, in0=t_b, scalar=-1.0, in1=t_a, op0=ALU.mult, op1=ALU.add)
            for c in range(3):
                nc.sync.dma_start(out=C1s[32 * c: 32 * c + bpc, :], in_=Cs[32 * c: 32 * c + bpc, 1: 1 + np_])

            XR = g16("XR", np_)
            nc.vector.tensor_tensor(out=XR, in0=Cs[:, 0:np_], in1=C1s, op=ALU.mult)
            YR = g16("YR", np_)
            nc.vector.tensor_tensor(out=YR, in0=Ds[:, 0:np_], in1=C1s, op=ALU.mult)

            def selmm(dst, src, count):
                for c0 in range(0, count, W):
                    n = min(W, count - c0)
                    nc.tensor.matmul(dst[:, c0 // W, :n], s48, src[:, c0: c0 + n], start=True, stop=True)

            lsq_ps = geo_ps.tile([16, NCH, W], dt.float32, name="lsq_ps", tag="gps")
            selmm(lsq_ps, DSQ, nl)
            lnl = geo.tile([16, nl], dt.float16, name="lnl")
            nc.scalar.activation(out=lnl, in_=lsq_ps.rearrange("p a b -> p (a b)")[:, 0:nl], func=AF.Ln)
            rlen = geo.tile([16, nl], dt.float16, name="rlen")
            nc.scalar.activation(out=rlen, in_=lnl, func=AF.Exp, scale=-0.5)
            lenf = geo.tile([16, nl], dt.float16, name="lenf")
            nc.scalar.activation(out=lenf, in_=lnl, func=AF.Exp, scale=0.5)
            # len feature: |d| - 3.8 (b1_fl adjusted host-side)
            nc.vector.tensor_scalar(out=F[64:80, 0:nl], in0=lenf, scalar1=3.8, scalar2=None, op0=ALU.subtract)

            rlen1 = geo.tile([16, nt], dt.float16, name="rlen1")
            nc.sync.dma_start(out=rlen1, in_=rlen[:, 1: 1 + nt])
            lenf1 = geo.tile([16, np_], dt.float16, name="lenf1")
            nc.sync.dma_start(out=lenf1, in_=lenf[:, 1: 1 + np_])

            dot_ps = geo_ps.tile([16, NCH, W], dt.float32, name="dot_ps", tag="gps")
            selmm(dot_ps, DD, nt)
            tt1 = geo.tile([16, nt], dt.float16, name="tt1")
            nc.vector.tensor_tensor(out=tt1, in0=dot_ps.rearrange("p a b -> p (a b)")[:, 0:nt], in1=rlen[:, 0:nt], op=ALU.mult)
            # cos(theta) = -(d.d') * rlen_i * rlen_{i+1}
            nc.vector.scalar_tensor_tensor(out=F[96:112, 0:nt], in0=tt1, scalar=-1.0, in1=rlen1, op0=ALU.mult, op1=ALU.mult)

            xr_ps = geo_ps.tile([16, NCH, W], dt.float32, name="xr_ps", tag="gps")
            selmm(xr_ps, XR, np_)
            x_sb = geo.tile([16, np_], dt.float16, name="x_sb")
            nc.scalar.activation(out=x_sb, in_=xr_ps.rearrange("p a b -> p (a b)")[:, 0:np_], func=AF.Copy)
            yr_ps = geo_ps.tile([16, NCH, W], dt.float32, name="yr_ps", tag="gps")
            selmm(yr_ps, YR, np_)
            y_sb = geo.tile([16, np_], dt.float16, name="y_sb")
            # y = (b1 . n2) * |b2|, extra 1/16 matches the s^4 scale of x
            nc.vector.scalar_tensor_tensor(
                out=y_sb, in0=yr_ps.rearrange("p a b -> p (a b)")[:, 0:np_],
                scalar=SINV, in1=lenf1, op0=ALU.mult, op1=ALU.mult)

            q1 = geo.tile([16, np_], dt.float16, name="q1")
            nc.vector.tensor_tensor(out=q1, in0=x_sb, in1=x_sb, op=ALU.mult)
            q2 = geo.tile([16, np_], dt.float16, name="q2")
            nc.vector.tensor_tensor(out=q2, in0=y_sb, in1=y_sb, op=ALU.mult)
            q = geo.tile([16, np_], dt.float16, name="q")
            nc.vector.tensor_tensor(out=q, in0=q1, in1=q2, op=ALU.add)
            lnq = geo.tile([16, np_], dt.float16, name="lnq")
            nc.scalar.activation(out=lnq, in_=q, func=AF.Ln, bias=eps_t)
            r2v = geo.tile([16, np_], dt.float16, name="r2v")
            nc.scalar.activation(out=r2v, in_=lnq, func=AF.Exp, scale=-0.5)
            # sin' = -sin(phi): sign folded into W1 sin rows host-side
            nc.vector.tensor_tensor(out=F[0:16, 0:np_], in0=y_sb, in1=r2v, op=ALU.mult)
            nc.vector.tensor_tensor(out=F[32:48, 0:np_], in0=x_sb, in1=r2v, op=ALU.mult)

        # ---------------- Phase 2: MLPs ----------------
        with tc.tile_pool(name="h1_ps", bufs=2, space="PSUM") as h1_ps, \
             tc.tile_pool(name="h1r_p", bufs=13) as h1r_p, \
             tc.tile_pool(name="h2_ps", bufs=2, space="PSUM") as h2_ps, \
             tc.tile_pool(name="scr_p", bufs=2) as scr_p:

            h1r_ref = {}

            def emit_w1(s, m):
                stack = stack_tiles[s]
                for h in range(2):
                    h1 = h1_ps.tile([H, 2, W], dt.float32, name="h1", tag="h1ps")
                    for ci in range(2):
                        c0 = (2 * h + ci) * W
                        n = min(W, (ll - KOFF[m]) - c0)
                        nc.tensor.matmul(h1[:, ci, :n], w1[m], stack[:, c0: c0 + n], start=True, stop=True)
                    h1r = h1r_p.tile([H, 2, W], dt.float16, name="h1r", tag="h1r")
                    nc.vector.tensor_scalar(
                        out=h1r.rearrange("p a b -> p (a b)"),
                        in0=h1.rearrange("p a b -> p (a b)"),
                        scalar1=0.0, scalar2=None, op0=ALU.max)
                    h1r_ref[(s, m, h)] = h1r

            def emit_w2(s, m):
                nv = ll - KOFF[m]
                for hh in range(2):
                    h2 = h2_ps.tile([H, 2, W], dt.float32, name="h2", tag="h2ps")
                    for ci in range(2):
                        c = 2 * hh + ci
                        n = min(W, nv - c * W)
                        h1r = h1r_ref[(s, m, hh)]
                        nc.tensor.matmul(h2[:, ci, :n], w2[m], h1r[:, ci, :n], start=True, stop=True)
                    nh = min(2 * W, nv - hh * 2 * W)
                    scr = scr_p.tile([H, 2, W], dt.float16, name="scr", tag="scr")
                    nc.scalar.activation(
                        out=scr.rearrange("p a b -> p (a b)")[:, 0:nh],
                        in_=h2.rearrange("p a b -> p (a b)")[:, 0:nh],
                        func=AF.Relu, bias=b2c[m],
                        accum_out=acc[m][:, 2 * s + hh: 2 * s + hh + 1])

            def prep_sample(s):
                if s not in stack_tiles:
                    alloc_stack(s)
                if s + 2 < bpc and (s + 2) not in stack_tiles:
                    alloc_stack(s + 2)     # keep the SE prefetch 2 samples ahead
                stack = stack_tiles[s]
                Fv = F.rearrange("(f s) l -> f s l", s=32)
                nc.sync.dma_start(out=stack[65:69, :], in_=Fv[:, s, :])

            for s in range(bpc + 1):
                if s < bpc:
                    prep_sample(s)
                for m in MLPS:
                    if s < bpc:
                        emit_w1(s, m)
                    if s >= 1:
                        emit_w2(s - 1, m)

        # ---------------- final reduction ----------------
        with tc.tile_pool(name="fin_ps", bufs=1, space="PSUM") as fin_ps:
            ep = fin_ps.tile([1, 3, 2 * bpc], dt.float32, name="ep")
            for j, m in enumerate(MLPS):
                nc.tensor.matmul(ep[:, j, :], w3c[m], acc[m], start=True, stop=True)
            esum = consts.tile([1, bpc], dt.float32, name="esum")
            nc.vector.tensor_reduce(
                out=esum, in_=ep.rearrange("o m (s h) -> o s m h", h=2), axis=AX.XY, op=ALU.add)
            eout = consts.tile([1, bpc], dt.float32, name="eout")
            nc.vector.tensor_scalar(out=eout, in0=esum, scalar1=b3s, scalar2=None, op0=ALU.add)
            nc.sync.dma_start(out=out_d.ap(), in_=eout)

    nc.finalize()
    return nc


_NC_CACHE = {}


def get_nc(bpc=BPC, ll=L):
    key = (bpc, ll)
    if key not in _NC_CACHE:
        _NC_CACHE[key] = build_nc(bpc, ll)
    return _NC_CACHE[key]


def _sel48():
    S = np.zeros((96, 16), np.float16)
    for c in range(3):
        for s in range(16):
            S[32 * c + s, s] = 1.0
    return S


def pack_weights(inputs):
    """Pack per-MLP W1 into the [69, H] stack-row layout (fp16), fold b1 via
    the ones row, flip the torsion sin-row sign, fold the 3.8 len-centering
    into b1_fl."""
    f32 = lambda k: np.asarray(inputs[k], np.float32)
    W1P = np.zeros((3, KROWS, H), np.float32)
    # fl: x = [len, e0, e1]
    w = f32("fl_W1")
    W1P[0, 0:32] = w[1:33]
    W1P[0, 67] = w[0]
    W1P[0, 64] = f32("fl_b1") + 3.8 * w[0]
    # ft: x = [cos_t, e0, e1, e2]
    w = f32("ft_W1")
    W1P[1, 0:48] = w[1:49]
    W1P[1, 68] = w[0]
    W1P[1, 64] = f32("ft_b1")
    # fp: x = [sin, cos, e0, e1, e2, e3]
    w = f32("fp_W1")
    W1P[2, 0:64] = w[2:66]
    W1P[2, 65] = -w[0]          # device computes -sin
    W1P[2, 66] = w[1]
    W1P[2, 64] = f32("fp_b1")
    W2P = np.stack([f32(f"{m}_W2") for m in MLPS]).astype(np.float16)
    B2P = np.stack([f32(f"{m}_b2").reshape(H, 1) for m in MLPS]).astype(np.float32)
    W3P = np.stack([f32(f"{m}_W3") for m in MLPS]).astype(np.float32)
    b3sum = np.float32(
        float(np.asarray(inputs["fl_b3"]).reshape(-1)[0]) * NL
        + float(np.asarray(inputs["ft_b3"]).reshape(-1)[0]) * NT
        + float(np.asarray(inputs["fp_b3"]).reshape(-1)[0]) * NP
    )
    return W1P.astype(np.float16), W2P, B2P, W3P, np.array([[b3sum]], np.float32)


def make_in_maps(inputs, bpc=BPC, ncores=NCORES):
    W1P, W2P, B2P, W3P, B3S = pack_weights(inputs)
    WALL = np.zeros((H, 6 * H), np.float16)
    for j in range(3):
        WALL[0:KROWS, H * j: H * (j + 1)] = W1P[j]
        WALL[:, H * (3 + j): H * (4 + j)] = W2P[j]
    BW = np.zeros((H, 8), np.float32)
    for j in range(3):
        BW[:, j] = B2P[j][:, 0]
        BW[:, 3 + j] = W3P[j][:, 0]
    BW[0, 6] = B3S[0, 0]
    emb16 = np.asarray(inputs["emb"], np.float32).astype(np.float16)
    seq = np.asarray(inputs["seq"], np.int64)
    R = np.asarray(inputs["R"], np.float32)
    e_all = emb16[seq]                       # [B, L, E]
    consts = dict(S48=_sel48(), WALL=WALL, BW=BW)
    in_maps = []
    for c in range(ncores):
        sl = slice(c * bpc, (c + 1) * bpc)
        Rt = np.ascontiguousarray(R[sl].transpose(2, 0, 1))        # [3, bpc, L]
        e = e_all[sl]                                              # [bpc, L, E]
        SE = np.zeros((bpc, 65, L), np.float16)
        for j in range(4):
            SE[:, 16 * j: 16 * j + 16, : L - j] = e[:, j:, :].transpose(0, 2, 1)
        SE[:, 64, :] = 1.0
        m = dict(consts)
        m["Rt"] = Rt
        m["SE"] = SE
        in_maps.append(m)
    return in_maps


def kernel(**inputs):
    nc = get_nc()
    in_maps = make_in_maps(inputs)
    res = bass_utils.run_bass_kernel_spmd(nc, in_maps, core_ids=list(range(NCORES)))
    return np.concatenate([res.results[c]["out"][0] for c in range(NCORES)]).astype(np.float32)


# revision 14
# speedup vs baseline: 2.6628x; 1.0543x over previous
"""Trainium2 Bass kernel for nn_LocalEnergy (protein local-energy GNN).

kernel(**inputs) takes FULL unsharded inputs (B=128), shards B across 8
NeuronCores (16 samples/core, pure data parallel), runs one Bass kernel
SPMD, gathers per-core [16] energies into the full [128] output.

v2 layout:
 - Host prep (indexing/layout only): embedding gather emb[seq] replicated
   into 4 shifted row-blocks + ones row -> SE [16, 65, 2048] fp16 per core;
   R transposed to [3, 16, L]; W1 packed (zero-padded, bias folded via the
   ones row, torsion sin-row sign-flipped) to match the on-device stack
   row layout.
 - Device phase 1 (geometry): fp16 vector math, rotations/shifts via DMA
   (no gpsimd), fp16 selection matmuls, Ln/Exp on scalar engine. Produces
   feature tile F [64, L] fp16 = [sin | cos | len-3.8 | cos_theta] blocks.
 - Device phase 2 (MLPs): per sample, stack [69, L] fp16 = SE rows + F
   rows; 3x fused W1 matmuls (K=69) per 512-chunk, relu on vector engine
   per 1024-col half, W2 matmuls, and one scalar-engine Relu+accumulate
   over all 2047-ish columns per (sample, MLP).
"""

import sys
import types
import numpy as np
from contextlib import ExitStack


def ensure_axon_hooks():
    """The container's antenv is a stub without axon_hooks; inject it so
    run_bass_kernel_spmd(trace=True) can NTFF-profile."""
    if "antenv.axon_hooks" in sys.modules:
        return
    import antenv

    hooks = types.ModuleType("antenv.axon_hooks")
    hooks._h = None

    def set_axon_ntff_profile_hook(h):
        hooks._h = h

    def get_axon_ntff_profile_hook():
        return hooks._h

    hooks.set_axon_ntff_profile_hook = set_axon_ntff_profile_hook
    hooks.get_axon_ntff_profile_hook = get_axon_ntff_profile_hook
    sys.modules["antenv.axon_hooks"] = hooks
    antenv.axon_hooks = hooks
    try:
        from trn_agent_boot.trn_boot import _ntff_profile_via_ctypes

        hook = _ntff_profile_via_ctypes("/opt/axon/libaxon_pjrt.so")
        if hook is not None:
            set_axon_ntff_profile_hook(hook)
    except Exception:
        pass


ensure_axon_hooks()

import concourse.bass as bass  # noqa: E402
import concourse.tile as tile  # noqa: E402
from concourse import mybir, bacc, bass_utils  # noqa: E402

dt = mybir.dt
AF = mybir.ActivationFunctionType
ALU = mybir.AluOpType
AX = mybir.AxisListType

NCORES = 8
B, L, NAA, E, H = 128, 2048, 20, 16, 128
BPC = B // NCORES
W = 512
NCH = L // W                       # 4 chunks of 512 per sample
NL, NT, NP = L - 1, L - 2, L - 3
KROWS = 69                         # stack rows: 64 emb-shift + ones + 4 features
SINV = 1.0 / 16.0                  # cross-product scaling to stay in fp16 range

MLPS = ("fl", "ft", "fp")
KOFF = {"fl": 1, "ft": 2, "fp": 3}   # valid cols per sample = L - KOFF


def build_nc(bpc=BPC, ll=L):
    nc = bacc.Bacc("TRN2", target_bir_lowering=False, debug=False)

    Rt_d = nc.dram_tensor("Rt", (3, bpc, ll), dt.float32, kind="ExternalInput")
    SE_d = nc.dram_tensor("SE", (bpc, 65, ll), dt.float16, kind="ExternalInput")
    S48_d = nc.dram_tensor("S48", (48, 16), dt.float16, kind="ExternalInput")
    WW_d = nc.dram_tensor("WALL", (H, 6 * H), dt.float16, kind="ExternalInput")
    BW_d = nc.dram_tensor("BW", (H, 8), dt.float32, kind="ExternalInput")
    out_d = nc.dram_tensor("out", (1, bpc), dt.float32, kind="ExternalOutput")

    nl, nt, np_ = ll - 1, ll - 2, ll - 3

    with tile.TileContext(nc) as tc, ExitStack() as ctx:
        consts = ctx.enter_context(tc.tile_pool(name="consts", bufs=1))
        # rt first: phase 1 is gated on it, so its DMAs lead the sync queue
        rt = consts.tile([48, ll], dt.float32, name="rt")
        for c in range(3):
            nc.sync.dma_start(out=rt[16 * c: 16 * c + bpc, :], in_=Rt_d.ap()[c])
        s48 = consts.tile([48, 16], dt.float16)
        nc.sync.dma_start(out=s48, in_=S48_d.ap())
        wall = consts.tile([H, 6 * H], dt.float16, name="wall")
        nc.sync.dma_start(out=wall, in_=WW_d.ap())
        bw = consts.tile([H, 8], dt.float32, name="bw")
        nc.sync.dma_start(out=bw, in_=BW_d.ap())
        w1, w2, b2c, w3c = {}, {}, {}, {}
        for j, m in enumerate(MLPS):
            w1[m] = wall[0:KROWS, H * j: H * (j + 1)]
            w2[m] = wall[:, H * (3 + j): H * (4 + j)]
            b2c[m] = bw[:, j: j + 1]
            w3c[m] = bw[:, 3 + j: 4 + j]
        b3s = bw[0:1, 6:7]

        # feature tile: rows 32f+s, f = 0:sin' 1:cos(phi) 2:len-3.8 3:cos(theta)
        F = consts.tile([128, ll], dt.float16, name="F")
        nc.vector.memset(F, 0.0)
        eps_t = consts.tile([16, 1], dt.float32, name="eps_t")
        nc.vector.memset(eps_t, 1e-6)
        lnsinv_t = consts.tile([16, 1], dt.float32, name="lnsinv_t")
        nc.vector.memset(lnsinv_t, float(np.log(SINV)))

        acc = {}
        for m in MLPS:
            acc[m] = consts.tile([H, 2 * bpc], dt.float32, name=f"acc_{m}")

        # stack pool opens before phase 1 so SE loads prefetch under geometry
        stk = ctx.enter_context(tc.tile_pool(name="stk", bufs=4))
        stack_tiles = {}

        def alloc_stack(s):
            t = stk.tile([KROWS, ll], dt.float16, name="stack", tag="stk")
            nc.sync.dma_start(out=t[0:65, :], in_=SE_d.ap()[s])
            stack_tiles[s] = t

        for s in range(4):
            alloc_stack(s)

        # ---------------- Phase 1: geometry ----------------
        with tc.tile_pool(name="geo", bufs=1) as geo, \
             tc.tile_pool(name="geo_ps", bufs=2, space="PSUM") as geo_ps:
            def g16(name, cols):
                return geo.tile([48, cols], dt.float16, name=name)

            A1 = g16("A1", nt)
            A2 = g16("A2", nt)
            B1 = g16("B1", nt)
            B2 = g16("B2", nt)
            C1s = g16("C1s", np_)

            D = g16("D", nl)
            nc.vector.tensor_tensor(out=D, in0=rt[:, 1:ll], in1=rt[:, 0:nl], op=ALU.subtract)
            D1 = g16("D1", nt)
            nc.vector.tensor_tensor(out=D1, in0=rt[:, 2:ll], in1=rt[:, 1:nl], op=ALU.subtract)
            Ds = g16("Ds", nl)
            nc.vector.tensor_scalar(out=Ds, in0=D, scalar1=SINV, scalar2=None, op0=ALU.mult)
            D1s = g16("D1s", nt)
            nc.vector.tensor_scalar(out=D1s, in0=D1, scalar1=SINV, scalar2=None, op0=ALU.mult)

            DSQ = g16("DSQ", nl)
            nc.vector.tensor_tensor(out=DSQ, in0=D, in1=D, op=ALU.mult)
            DD = g16("DD", nt)
            nc.vector.tensor_tensor(out=DD, in0=D[:, 0:nt], in1=D1, op=ALU.mult)

            # coordinate rotations of the scaled bond vectors, via DMA
            for c in range(3):
                c1, c2 = (c + 1) % 3, (c + 2) % 3
                nc.sync.dma_start(out=A1[16 * c: 16 * c + bpc, :], in_=Ds[16 * c1: 16 * c1 + bpc, 0:nt])
                nc.sync.dma_start(out=A2[16 * c: 16 * c + bpc, :], in_=Ds[16 * c2: 16 * c2 + bpc, 0:nt])
                nc.sync.dma_start(out=B1[16 * c: 16 * c + bpc, :], in_=D1s[16 * c1: 16 * c1 + bpc, :])
                nc.sync.dma_start(out=B2[16 * c: 16 * c + bpc, :], in_=D1s[16 * c2: 16 * c2 + bpc, :])

            t_a = g16("t_a", nt)
            nc.vector.tensor_tensor(out=t_a, in0=A1, in1=B2, op=ALU.mult)
            t_b = g16("t_b", nt)
            nc.vector.tensor_tensor(out=t_b, in0=A2, in1=B1, op=ALU.mult)
            Cs = g16("Cs", nt)
            nc.vector.tensor_tensor(out=Cs, in0=t_a, in1=t_b, op=ALU.subtract)
            for c in range(3):
                nc.sync.dma_start(out=C1s[16 * c: 16 * c + bpc, :], in_=Cs[16 * c: 16 * c + bpc, 1: 1 + np_])

            XR = g16("XR", np_)
            nc.vector.tensor_tensor(out=XR, in0=Cs[:, 0:np_], in1=C1s, op=ALU.mult)
            YR = g16("YR", np_)
            nc.vector.tensor_tensor(out=YR, in0=Ds[:, 0:np_], in1=C1s, op=ALU.mult)

            def selmm(dst, src, count):
                for c0 in range(0, count, W):
                    n = min(W, count - c0)
                    nc.tensor.matmul(dst[:, c0 // W, :n], s48, src[:, c0: c0 + n], start=True, stop=True)

            lsq_ps = geo_ps.tile([16, NCH, W], dt.float32, name="lsq_ps", tag="gps")
            selmm(lsq_ps, DSQ, nl)
            lnl = geo.tile([16, nl], dt.float16, name="lnl")
            nc.scalar.activation(out=lnl, in_=lsq_ps.rearrange("p a b -> p (a b)")[:, 0:nl], func=AF.Ln)
            rlen = geo.tile([16, nl], dt.float16, name="rlen")
            nc.scalar.activation(out=rlen, in_=lnl, func=AF.Exp, scale=-0.5)
            lenf = geo.tile([16, nl], dt.float16, name="lenf")
            nc.scalar.activation(out=lenf, in_=lnl, func=AF.Exp, scale=0.5)
            lenf16 = geo.tile([16, nl], dt.float16, name="lenf16")
            nc.scalar.activation(out=lenf16, in_=lnl, func=AF.Exp, scale=0.5, bias=lnsinv_t)
            # len feature: |d| - 3.8 (b1_fl adjusted host-side)
            nc.vector.tensor_scalar(out=F[64:80, 0:nl], in0=lenf, scalar1=3.8, scalar2=None, op0=ALU.subtract)

            rlen1 = geo.tile([16, nt], dt.float16, name="rlen1")
            nc.sync.dma_start(out=rlen1, in_=rlen[:, 1: 1 + nt])
            lenf1 = geo.tile([16, np_], dt.float16, name="lenf1")
            nc.sync.dma_start(out=lenf1, in_=lenf16[:, 1: 1 + np_])

            dot_ps = geo_ps.tile([16, NCH, W], dt.float32, name="dot_ps", tag="gps")
            selmm(dot_ps, DD, nt)
            tt1 = geo.tile([16, nt], dt.float16, name="tt1")
            nc.vector.tensor_tensor(out=tt1, in0=dot_ps.rearrange("p a b -> p (a b)")[:, 0:nt], in1=rlen[:, 0:nt], op=ALU.mult)
            # store +(d.d')*rlen_i*rlen_{i+1} = -cos(theta); sign folded into W1_ft
            nc.vector.tensor_tensor(out=F[96:112, 0:nt], in0=tt1, in1=rlen1, op=ALU.mult)

            xr_ps = geo_ps.tile([16, NCH, W], dt.float32, name="xr_ps", tag="gps")
            selmm(xr_ps, XR, np_)
            x_sb = geo.tile([16, np_], dt.float16, name="x_sb")
            nc.scalar.activation(out=x_sb, in_=xr_ps.rearrange("p a b -> p (a b)")[:, 0:np_], func=AF.Copy)
            yr_ps = geo_ps.tile([16, NCH, W], dt.float32, name="yr_ps", tag="gps")
            selmm(yr_ps, YR, np_)
            y_sb = geo.tile([16, np_], dt.float16, name="y_sb")
            # y = (b1 . n2) * |b2| / 16 (the 1/16 rides in lenf16) -> matches x's s^4 scale
            nc.vector.tensor_tensor(
                out=y_sb, in0=yr_ps.rearrange("p a b -> p (a b)")[:, 0:np_],
                in1=lenf1, op=ALU.mult)

            q1 = geo.tile([16, np_], dt.float16, name="q1")
            nc.vector.tensor_tensor(out=q1, in0=x_sb, in1=x_sb, op=ALU.mult)
            q2 = geo.tile([16, np_], dt.float16, name="q2")
            nc.vector.tensor_tensor(out=q2, in0=y_sb, in1=y_sb, op=ALU.mult)
            q = geo.tile([16, np_], dt.float16, name="q")
            nc.vector.tensor_tensor(out=q, in0=q1, in1=q2, op=ALU.add)
            lnq = geo.tile([16, np_], dt.float16, name="lnq")
            nc.scalar.activation(out=lnq, in_=q, func=AF.Ln, bias=eps_t)
            r2v = geo.tile([16, np_], dt.float16, name="r2v")
            nc.scalar.activation(out=r2v, in_=lnq, func=AF.Exp, scale=-0.5)
            # sin' = -sin(phi): sign folded into W1 sin rows host-side
            nc.vector.tensor_tensor(out=F[0:16, 0:np_], in0=y_sb, in1=r2v, op=ALU.mult)
            nc.vector.tensor_tensor(out=F[32:48, 0:np_], in0=x_sb, in1=r2v, op=ALU.mult)

        # ---------------- Phase 2: MLPs ----------------
        with tc.tile_pool(name="h1_ps", bufs=2, space="PSUM") as h1_ps, \
             tc.tile_pool(name="h1r_p", bufs=13) as h1r_p, \
             tc.tile_pool(name="h2_ps", bufs=2, space="PSUM") as h2_ps, \
             tc.tile_pool(name="scr_p", bufs=2) as scr_p:

            h1r_ref = {}

            def emit_w1(s, m):
                stack = stack_tiles[s]
                for h in range(2):
                    h1 = h1_ps.tile([H, 2, W], dt.float32, name="h1", tag="h1ps")
                    for ci in range(2):
                        c0 = (2 * h + ci) * W
                        n = min(W, (ll - KOFF[m]) - c0)
                        nc.tensor.matmul(h1[:, ci, :n], w1[m], stack[:, c0: c0 + n], start=True, stop=True)
                    h1r = h1r_p.tile([H, 2, W], dt.float16, name="h1r", tag="h1r")
                    nc.vector.tensor_scalar(
                        out=h1r.rearrange("p a b -> p (a b)"),
                        in0=h1.rearrange("p a b -> p (a b)"),
                        scalar1=0.0, scalar2=None, op0=ALU.max)
                    h1r_ref[(s, m, h)] = h1r

            def emit_w2(s, m):
                nv = ll - KOFF[m]
                for hh in range(2):
                    h2 = h2_ps.tile([H, 2, W], dt.float32, name="h2", tag="h2ps")
                    for ci in range(2):
                        c = 2 * hh + ci
                        n = min(W, nv - c * W)
                        h1r = h1r_ref[(s, m, hh)]
                        nc.tensor.matmul(h2[:, ci, :n], w2[m], h1r[:, ci, :n], start=True, stop=True)
                    nh = min(2 * W, nv - hh * 2 * W)
                    scr = scr_p.tile([H, 2, W], dt.float16, name="scr", tag="scr")
                    nc.scalar.activation(
                        out=scr.rearrange("p a b -> p (a b)")[:, 0:nh],
                        in_=h2.rearrange("p a b -> p (a b)")[:, 0:nh],
                        func=AF.Relu, bias=b2c[m],
                        accum_out=acc[m][:, 2 * s + hh: 2 * s + hh + 1])

            def prep_sample(s):
                if s not in stack_tiles:
                    alloc_stack(s)
                if s + 2 < bpc and (s + 2) not in stack_tiles:
                    alloc_stack(s + 2)     # keep the SE prefetch 2 samples ahead
                stack = stack_tiles[s]
                Fv = F.rearrange("(f s) l -> f s l", s=32)
                nc.sync.dma_start(out=stack[65:69, :], in_=Fv[:, s, :])

            for s in range(bpc + 1):
                if s < bpc:
                    prep_sample(s)
                for m in MLPS:
                    if s < bpc:
                        emit_w1(s, m)
                    if s >= 1:
                        emit_w2(s - 1, m)

        # ---------------- final reduction ----------------
        with tc.tile_pool(name="fin_ps", bufs=1, space="PSUM") as fin_ps:
            ep = fin_ps.tile([1, 3, 2 * bpc], dt.float32, name="ep")
            for j, m in enumerate(MLPS):
                nc.tensor.matmul(ep[:, j, :], w3c[m], acc[m], start=True, stop=True)
            esum = consts.tile([1, bpc], dt.float32, name="esum")
            nc.vector.tensor_reduce(
                out=esum, in_=ep.rearrange("o m (s h) -> o s m h", h=2), axis=AX.XY, op=ALU.add)
            eout = consts.tile([1, bpc], dt.float32, name="eout")
            nc.vector.tensor_scalar(out=eout, in0=esum, scalar1=b3s, scalar2=None, op0=ALU.add)
            nc.sync.dma_start(out=out_d.ap(), in_=eout)

    nc.finalize()
    return nc


_NC_CACHE = {}


def get_nc(bpc=BPC, ll=L):
    key = (bpc, ll)
    if key not in _NC_CACHE:
        _NC_CACHE[key] = build_nc(bpc, ll)
    return _NC_CACHE[key]


def _sel48():
    S = np.zeros((48, 16), np.float16)
    for c in range(3):
        for s in range(16):
            S[16 * c + s, s] = 1.0
    return S


def pack_weights(inputs):
    """Pack per-MLP W1 into the [69, H] stack-row layout (fp16), fold b1 via
    the ones row, flip the torsion sin-row sign, fold the 3.8 len-centering
    into b1_fl."""
    f32 = lambda k: np.asarray(inputs[k], np.float32)
    W1P = np.zeros((3, KROWS, H), np.float32)
    # fl: x = [len, e0, e1]
    w = f32("fl_W1")
    W1P[0, 0:32] = w[1:33]
    W1P[0, 67] = w[0]
    W1P[0, 64] = f32("fl_b1") + 3.8 * w[0]
    # ft: x = [cos_t, e0, e1, e2]
    w = f32("ft_W1")
    W1P[1, 0:48] = w[1:49]
    W1P[1, 68] = -w[0]          # device stores -cos(theta)
    W1P[1, 64] = f32("ft_b1")
    # fp: x = [sin, cos, e0, e1, e2, e3]
    w = f32("fp_W1")
    W1P[2, 0:64] = w[2:66]
    W1P[2, 65] = -w[0]          # device computes -sin
    W1P[2, 66] = w[1]
    W1P[2, 64] = f32("fp_b1")
    W2P = np.stack([f32(f"{m}_W2") for m in MLPS]).astype(np.float16)
    B2P = np.stack([f32(f"{m}_b2").reshape(H, 1) for m in MLPS]).astype(np.float32)
    W3P = np.stack([f32(f"{m}_W3") for m in MLPS]).astype(np.float32)
    b3sum = np.float32(
        float(np.asarray(inputs["fl_b3"]).reshape(-1)[0]) * NL
        + float(np.asarray(inputs["ft_b3"]).reshape(-1)[0]) * NT
        + float(np.asarray(inputs["fp_b3"]).reshape(-1)[0]) * NP
    )
    return W1P.astype(np.float16), W2P, B2P, W3P, np.array([[b3sum]], np.float32)


def make_in_maps(inputs, bpc=BPC, ncores=NCORES):
    W1P, W2P, B2P, W3P, B3S = pack_weights(inputs)
    WALL = np.zeros((H, 6 * H), np.float16)
    for j in range(3):
        WALL[0:KROWS, H * j: H * (j + 1)] = W1P[j]
        WALL[:, H * (3 + j): H * (4 + j)] = W2P[j]
    BW = np.zeros((H, 8), np.float32)
    for j in range(3):
        BW[:, j] = B2P[j][:, 0]
        BW[:, 3 + j] = W3P[j][:, 0]
    BW[0, 6] = B3S[0, 0]
    emb16 = np.asarray(inputs["emb"], np.float32).astype(np.float16)
    seq = np.asarray(inputs["seq"], np.int64)
    R = np.asarray(inputs["R"], np.float32)
    e_all = emb16[seq]                       # [B, L, E]
    consts = dict(S48=_sel48(), WALL=WALL, BW=BW)
    in_maps = []
    for c in range(ncores):
        sl = slice(c * bpc, (c + 1) * bpc)
        Rt = np.ascontiguousarray(R[sl].transpose(2, 0, 1))        # [3, bpc, L]
        e = e_all[sl]                                              # [bpc, L, E]
        SE = np.zeros((bpc, 65, L), np.float16)
        for j in range(4):
            SE[:, 16 * j: 16 * j + 16, : L - j] = e[:, j:, :].transpose(0, 2, 1)
        SE[:, 64, :] = 1.0
        m = dict(consts)
        m["Rt"] = Rt
        m["SE"] = SE
        in_maps.append(m)
    return in_maps


def kernel(**inputs):
    nc = get_nc()
    in_maps = make_in_maps(inputs)
    res = bass_utils.run_bass_kernel_spmd(nc, in_maps, core_ids=list(range(NCORES)))
    return np.concatenate([res.results[c]["out"][0] for c in range(NCORES)]).astype(np.float32)


# revision 15
# speedup vs baseline: 2.7029x; 1.0150x over previous
"""Trainium2 Bass kernel for nn_LocalEnergy (protein local-energy GNN).

kernel(**inputs) takes FULL unsharded inputs (B=128), shards B across 8
NeuronCores (16 samples/core, pure data parallel), runs one Bass kernel
SPMD, gathers per-core [16] energies into the full [128] output.

v2 layout:
 - Host prep (indexing/layout only): embedding gather emb[seq] replicated
   into 4 shifted row-blocks + ones row -> SE [16, 65, 2048] fp16 per core;
   R transposed to [3, 16, L]; W1 packed (zero-padded, bias folded via the
   ones row, torsion sin-row sign-flipped) to match the on-device stack
   row layout.
 - Device phase 1 (geometry): fp16 vector math, rotations/shifts via DMA
   (no gpsimd), fp16 selection matmuls, Ln/Exp on scalar engine. Produces
   feature tile F [64, L] fp16 = [sin | cos | len-3.8 | cos_theta] blocks.
 - Device phase 2 (MLPs): per sample, stack [69, L] fp16 = SE rows + F
   rows; 3x fused W1 matmuls (K=69) per 512-chunk, relu on vector engine
   per 1024-col half, W2 matmuls, and one scalar-engine Relu+accumulate
   over all 2047-ish columns per (sample, MLP).
"""

import sys
import types
import numpy as np
from contextlib import ExitStack


def ensure_axon_hooks():
    """The container's antenv is a stub without axon_hooks; inject it so
    run_bass_kernel_spmd(trace=True) can NTFF-profile."""
    if "antenv.axon_hooks" in sys.modules:
        return
    import antenv

    hooks = types.ModuleType("antenv.axon_hooks")
    hooks._h = None

    def set_axon_ntff_profile_hook(h):
        hooks._h = h

    def get_axon_ntff_profile_hook():
        return hooks._h

    hooks.set_axon_ntff_profile_hook = set_axon_ntff_profile_hook
    hooks.get_axon_ntff_profile_hook = get_axon_ntff_profile_hook
    sys.modules["antenv.axon_hooks"] = hooks
    antenv.axon_hooks = hooks
    try:
        from trn_agent_boot.trn_boot import _ntff_profile_via_ctypes

        hook = _ntff_profile_via_ctypes("/opt/axon/libaxon_pjrt.so")
        if hook is not None:
            set_axon_ntff_profile_hook(hook)
    except Exception:
        pass


ensure_axon_hooks()

import concourse.bass as bass  # noqa: E402
import concourse.tile as tile  # noqa: E402
from concourse import mybir, bacc, bass_utils  # noqa: E402

dt = mybir.dt
AF = mybir.ActivationFunctionType
ALU = mybir.AluOpType
AX = mybir.AxisListType

NCORES = 8
B, L, NAA, E, H = 128, 2048, 20, 16, 128
BPC = B // NCORES
W = 512
NCH = L // W                       # 4 chunks of 512 per sample
NL, NT, NP = L - 1, L - 2, L - 3
KROWS = 69                         # stack rows: 64 emb-shift + ones + 4 features
SINV = 1.0 / 16.0                  # cross-product scaling to stay in fp16 range

MLPS = ("fl", "ft", "fp")
W1K = {"fl": 66, "ft": 67, "fp": 69}   # stack rows each MLP reads
KOFF = {"fl": 1, "ft": 2, "fp": 3}   # valid cols per sample = L - KOFF


def build_nc(bpc=BPC, ll=L):
    nc = bacc.Bacc("TRN2", target_bir_lowering=False, debug=False)

    Rt_d = nc.dram_tensor("Rt", (3, bpc, ll), dt.float32, kind="ExternalInput")
    SE_d = nc.dram_tensor("SE", (bpc, 65, ll), dt.float16, kind="ExternalInput")
    S48_d = nc.dram_tensor("S48", (48, 16), dt.float16, kind="ExternalInput")
    WW_d = nc.dram_tensor("WALL", (H, 6 * H), dt.float16, kind="ExternalInput")
    BW_d = nc.dram_tensor("BW", (H, 8), dt.float32, kind="ExternalInput")
    out_d = nc.dram_tensor("out", (1, bpc), dt.float32, kind="ExternalOutput")

    nl, nt, np_ = ll - 1, ll - 2, ll - 3

    with tile.TileContext(nc) as tc, ExitStack() as ctx:
        consts = ctx.enter_context(tc.tile_pool(name="consts", bufs=1))
        # rt first: phase 1 is gated on it, so its DMAs lead the sync queue
        rt = consts.tile([48, ll], dt.float32, name="rt")
        for c in range(3):
            nc.sync.dma_start(out=rt[16 * c: 16 * c + bpc, :], in_=Rt_d.ap()[c])
        s48 = consts.tile([48, 16], dt.float16)
        nc.sync.dma_start(out=s48, in_=S48_d.ap())
        wall = consts.tile([H, 6 * H], dt.float16, name="wall")
        nc.sync.dma_start(out=wall, in_=WW_d.ap())
        bw = consts.tile([H, 8], dt.float32, name="bw")
        nc.sync.dma_start(out=bw, in_=BW_d.ap())
        w1, w2, b2c, w3c = {}, {}, {}, {}
        for j, m in enumerate(MLPS):
            w1[m] = wall[0:W1K[m], H * j: H * (j + 1)]
            w2[m] = wall[:, H * (3 + j): H * (4 + j)]
            b2c[m] = bw[:, j: j + 1]
            w3c[m] = bw[:, 3 + j: 4 + j]
        b3s = bw[0:1, 6:7]

        # feature tile: rows 32f+s, f = 0:sin' 1:cos(phi) 2:len-3.8 3:cos(theta)
        F = consts.tile([128, ll], dt.float16, name="F")
        nc.vector.memset(F, 0.0)
        eps_t = consts.tile([16, 1], dt.float32, name="eps_t")
        nc.vector.memset(eps_t, 1e-6)
        lnsinv_t = consts.tile([16, 1], dt.float32, name="lnsinv_t")
        nc.vector.memset(lnsinv_t, float(np.log(SINV)))

        acc = {}
        for m in MLPS:
            acc[m] = consts.tile([H, 2 * bpc], dt.float32, name=f"acc_{m}")

        # stack pool opens before phase 1 so SE loads prefetch under geometry
        stk = ctx.enter_context(tc.tile_pool(name="stk", bufs=4))
        stack_tiles = {}

        def alloc_stack(s):
            t = stk.tile([KROWS, ll], dt.float16, name="stack", tag="stk")
            nc.sync.dma_start(out=t[0:65, :], in_=SE_d.ap()[s])
            stack_tiles[s] = t

        for s in range(4):
            alloc_stack(s)

        # ---------------- Phase 1: geometry ----------------
        with tc.tile_pool(name="geo", bufs=1) as geo, \
             tc.tile_pool(name="geo_ps", bufs=2, space="PSUM") as geo_ps:
            def g16(name, cols):
                return geo.tile([48, cols], dt.float16, name=name)

            A1 = g16("A1", nt)
            A2 = g16("A2", nt)
            B1 = g16("B1", nt)
            B2 = g16("B2", nt)
            C1s = g16("C1s", np_)

            D = g16("D", nl)
            nc.vector.tensor_tensor(out=D, in0=rt[:, 1:ll], in1=rt[:, 0:nl], op=ALU.subtract)
            D1 = g16("D1", nt)
            nc.vector.tensor_tensor(out=D1, in0=rt[:, 2:ll], in1=rt[:, 1:nl], op=ALU.subtract)
            Ds = g16("Ds", nl)
            nc.vector.tensor_scalar(out=Ds, in0=D, scalar1=SINV, scalar2=None, op0=ALU.mult)
            D1s = g16("D1s", nt)
            nc.vector.tensor_scalar(out=D1s, in0=D1, scalar1=SINV, scalar2=None, op0=ALU.mult)

            DSQ = g16("DSQ", nl)
            nc.vector.tensor_tensor(out=DSQ, in0=D, in1=D, op=ALU.mult)
            DD = g16("DD", nt)
            nc.vector.tensor_tensor(out=DD, in0=D[:, 0:nt], in1=D1, op=ALU.mult)

            # coordinate rotations of the scaled bond vectors, via DMA
            for c in range(3):
                c1, c2 = (c + 1) % 3, (c + 2) % 3
                nc.sync.dma_start(out=A1[16 * c: 16 * c + bpc, :], in_=Ds[16 * c1: 16 * c1 + bpc, 0:nt])
                nc.sync.dma_start(out=A2[16 * c: 16 * c + bpc, :], in_=Ds[16 * c2: 16 * c2 + bpc, 0:nt])
                nc.sync.dma_start(out=B1[16 * c: 16 * c + bpc, :], in_=D1s[16 * c1: 16 * c1 + bpc, :])
                nc.sync.dma_start(out=B2[16 * c: 16 * c + bpc, :], in_=D1s[16 * c2: 16 * c2 + bpc, :])

            t_a = g16("t_a", nt)
            nc.vector.tensor_tensor(out=t_a, in0=A1, in1=B2, op=ALU.mult)
            t_b = g16("t_b", nt)
            nc.vector.tensor_tensor(out=t_b, in0=A2, in1=B1, op=ALU.mult)
            Cs = g16("Cs", nt)
            nc.vector.tensor_tensor(out=Cs, in0=t_a, in1=t_b, op=ALU.subtract)
            for c in range(3):
                nc.sync.dma_start(out=C1s[16 * c: 16 * c + bpc, :], in_=Cs[16 * c: 16 * c + bpc, 1: 1 + np_])

            XR = g16("XR", np_)
            nc.vector.tensor_tensor(out=XR, in0=Cs[:, 0:np_], in1=C1s, op=ALU.mult)
            YR = g16("YR", np_)
            nc.vector.tensor_tensor(out=YR, in0=Ds[:, 0:np_], in1=C1s, op=ALU.mult)

            def selmm(dst, src, count):
                for c0 in range(0, count, W):
                    n = min(W, count - c0)
                    nc.tensor.matmul(dst[:, c0 // W, :n], s48, src[:, c0: c0 + n], start=True, stop=True)

            lsq_ps = geo_ps.tile([16, NCH, W], dt.float32, name="lsq_ps", tag="gps")
            selmm(lsq_ps, DSQ, nl)
            lnl = geo.tile([16, nl], dt.float16, name="lnl")
            nc.scalar.activation(out=lnl, in_=lsq_ps.rearrange("p a b -> p (a b)")[:, 0:nl], func=AF.Ln)
            rlen = geo.tile([16, nl], dt.float16, name="rlen")
            nc.scalar.activation(out=rlen, in_=lnl, func=AF.Exp, scale=-0.5)
            lenf = geo.tile([16, nl], dt.float16, name="lenf")
            nc.scalar.activation(out=lenf, in_=lnl, func=AF.Exp, scale=0.5)
            lenf16 = geo.tile([16, nl], dt.float16, name="lenf16")
            nc.scalar.activation(out=lenf16, in_=lnl, func=AF.Exp, scale=0.5, bias=lnsinv_t)
            # len feature: |d| - 3.8 (b1_fl adjusted host-side)
            nc.vector.tensor_scalar(out=F[0:16, 0:nl], in0=lenf, scalar1=3.8, scalar2=None, op0=ALU.subtract)

            rlen1 = geo.tile([16, nt], dt.float16, name="rlen1")
            nc.sync.dma_start(out=rlen1, in_=rlen[:, 1: 1 + nt])
            lenf1 = geo.tile([16, np_], dt.float16, name="lenf1")
            nc.sync.dma_start(out=lenf1, in_=lenf16[:, 1: 1 + np_])

            dot_ps = geo_ps.tile([16, NCH, W], dt.float32, name="dot_ps", tag="gps")
            selmm(dot_ps, DD, nt)
            tt1 = geo.tile([16, nt], dt.float16, name="tt1")
            nc.vector.tensor_tensor(out=tt1, in0=dot_ps.rearrange("p a b -> p (a b)")[:, 0:nt], in1=rlen[:, 0:nt], op=ALU.mult)
            # store +(d.d')*rlen_i*rlen_{i+1} = -cos(theta); sign folded into W1_ft
            nc.vector.tensor_tensor(out=F[32:48, 0:nt], in0=tt1, in1=rlen1, op=ALU.mult)

            xr_ps = geo_ps.tile([16, NCH, W], dt.float32, name="xr_ps", tag="gps")
            selmm(xr_ps, XR, np_)
            x_sb = geo.tile([16, np_], dt.float16, name="x_sb")
            nc.scalar.activation(out=x_sb, in_=xr_ps.rearrange("p a b -> p (a b)")[:, 0:np_], func=AF.Copy)
            yr_ps = geo_ps.tile([16, NCH, W], dt.float32, name="yr_ps", tag="gps")
            selmm(yr_ps, YR, np_)
            y_sb = geo.tile([16, np_], dt.float16, name="y_sb")
            # y = (b1 . n2) * |b2| / 16 (the 1/16 rides in lenf16) -> matches x's s^4 scale
            nc.vector.tensor_tensor(
                out=y_sb, in0=yr_ps.rearrange("p a b -> p (a b)")[:, 0:np_],
                in1=lenf1, op=ALU.mult)

            q1 = geo.tile([16, np_], dt.float16, name="q1")
            nc.vector.tensor_tensor(out=q1, in0=x_sb, in1=x_sb, op=ALU.mult)
            q2 = geo.tile([16, np_], dt.float16, name="q2")
            nc.vector.tensor_tensor(out=q2, in0=y_sb, in1=y_sb, op=ALU.mult)
            q = geo.tile([16, np_], dt.float16, name="q")
            nc.vector.tensor_tensor(out=q, in0=q1, in1=q2, op=ALU.add)
            lnq = geo.tile([16, np_], dt.float16, name="lnq")
            nc.scalar.activation(out=lnq, in_=q, func=AF.Ln, bias=eps_t)
            r2v = geo.tile([16, np_], dt.float16, name="r2v")
            nc.scalar.activation(out=r2v, in_=lnq, func=AF.Exp, scale=-0.5)
            # sin' = -sin(phi): sign folded into W1 sin rows host-side
            nc.vector.tensor_tensor(out=F[64:80, 0:np_], in0=y_sb, in1=r2v, op=ALU.mult)
            nc.vector.tensor_tensor(out=F[96:112, 0:np_], in0=x_sb, in1=r2v, op=ALU.mult)

        # ---------------- Phase 2: MLPs ----------------
        with tc.tile_pool(name="h1_ps", bufs=2, space="PSUM") as h1_ps, \
             tc.tile_pool(name="h1r_p", bufs=13) as h1r_p, \
             tc.tile_pool(name="h2_ps", bufs=2, space="PSUM") as h2_ps, \
             tc.tile_pool(name="scr_p", bufs=2) as scr_p:

            h1r_ref = {}

            def emit_w1(s, m):
                stack = stack_tiles[s]
                for h in range(2):
                    h1 = h1_ps.tile([H, 2, W], dt.float32, name="h1", tag="h1ps")
                    for ci in range(2):
                        c0 = (2 * h + ci) * W
                        n = min(W, (ll - KOFF[m]) - c0)
                        nc.tensor.matmul(h1[:, ci, :n], w1[m], stack[0:W1K[m], c0: c0 + n], start=True, stop=True)
                    h1r = h1r_p.tile([H, 2, W], dt.float16, name="h1r", tag="h1r")
                    nc.vector.tensor_scalar(
                        out=h1r.rearrange("p a b -> p (a b)"),
                        in0=h1.rearrange("p a b -> p (a b)"),
                        scalar1=0.0, scalar2=None, op0=ALU.max)
                    h1r_ref[(s, m, h)] = h1r

            def emit_w2(s, m):
                nv = ll - KOFF[m]
                for hh in range(2):
                    h2 = h2_ps.tile([H, 2, W], dt.float32, name="h2", tag="h2ps")
                    for ci in range(2):
                        c = 2 * hh + ci
                        n = min(W, nv - c * W)
                        h1r = h1r_ref[(s, m, hh)]
                        nc.tensor.matmul(h2[:, ci, :n], w2[m], h1r[:, ci, :n], start=True, stop=True)
                    nh = min(2 * W, nv - hh * 2 * W)
                    scr = scr_p.tile([H, 2, W], dt.float16, name="scr", tag="scr")
                    nc.scalar.activation(
                        out=scr.rearrange("p a b -> p (a b)")[:, 0:nh],
                        in_=h2.rearrange("p a b -> p (a b)")[:, 0:nh],
                        func=AF.Relu, bias=b2c[m],
                        accum_out=acc[m][:, 2 * s + hh: 2 * s + hh + 1])

            def prep_sample(s):
                if s not in stack_tiles:
                    alloc_stack(s)
                if s + 2 < bpc and (s + 2) not in stack_tiles:
                    alloc_stack(s + 2)     # keep the SE prefetch 2 samples ahead
                stack = stack_tiles[s]
                Fv = F.rearrange("(f s) l -> f s l", s=32)
                nc.sync.dma_start(out=stack[65:67, :], in_=Fv[0:2, s, :])
                nc.sync.dma_start(out=stack[67:69, :], in_=Fv[2:4, s, :])

            for s in range(bpc + 1):
                if s < bpc:
                    prep_sample(s)
                for m in MLPS:
                    if s < bpc:
                        emit_w1(s, m)
                    if s >= 1:
                        emit_w2(s - 1, m)

        # ---------------- final reduction ----------------
        with tc.tile_pool(name="fin_ps", bufs=1, space="PSUM") as fin_ps:
            ep = fin_ps.tile([1, 3, 2 * bpc], dt.float32, name="ep")
            for j, m in enumerate(MLPS):
                nc.tensor.matmul(ep[:, j, :], w3c[m], acc[m], start=True, stop=True)
            esum = consts.tile([1, bpc], dt.float32, name="esum")
            nc.vector.tensor_reduce(
                out=esum, in_=ep.rearrange("o m (s h) -> o s m h", h=2), axis=AX.XY, op=ALU.add)
            eout = consts.tile([1, bpc], dt.float32, name="eout")
            nc.vector.tensor_scalar(out=eout, in0=esum, scalar1=b3s, scalar2=None, op0=ALU.add)
            nc.sync.dma_start(out=out_d.ap(), in_=eout)

    nc.finalize()
    return nc


_NC_CACHE = {}


def get_nc(bpc=BPC, ll=L):
    key = (bpc, ll)
    if key not in _NC_CACHE:
        _NC_CACHE[key] = build_nc(bpc, ll)
    return _NC_CACHE[key]


def _sel48():
    S = np.zeros((48, 16), np.float16)
    for c in range(3):
        for s in range(16):
            S[16 * c + s, s] = 1.0
    return S


def pack_weights(inputs):
    """Pack per-MLP W1 into the [69, H] stack-row layout (fp16), fold b1 via
    the ones row, flip the torsion sin-row sign, fold the 3.8 len-centering
    into b1_fl."""
    f32 = lambda k: np.asarray(inputs[k], np.float32)
    W1P = np.zeros((3, KROWS, H), np.float32)
    # fl: x = [len, e0, e1]
    w = f32("fl_W1")
    W1P[0, 0:32] = w[1:33]
    W1P[0, 65] = w[0]
    W1P[0, 64] = f32("fl_b1") + 3.8 * w[0]
    # ft: x = [cos_t, e0, e1, e2]
    w = f32("ft_W1")
    W1P[1, 0:48] = w[1:49]
    W1P[1, 66] = -w[0]          # device stores -cos(theta)
    W1P[1, 64] = f32("ft_b1")
    # fp: x = [sin, cos, e0, e1, e2, e3]
    w = f32("fp_W1")
    W1P[2, 0:64] = w[2:66]
    W1P[2, 67] = -w[0]          # device computes -sin
    W1P[2, 68] = w[1]
    W1P[2, 64] = f32("fp_b1")
    W2P = np.stack([f32(f"{m}_W2") for m in MLPS]).astype(np.float16)
    B2P = np.stack([f32(f"{m}_b2").reshape(H, 1) for m in MLPS]).astype(np.float32)
    W3P = np.stack([f32(f"{m}_W3") for m in MLPS]).astype(np.float32)
    b3sum = np.float32(
        float(np.asarray(inputs["fl_b3"]).reshape(-1)[0]) * NL
        + float(np.asarray(inputs["ft_b3"]).reshape(-1)[0]) * NT
        + float(np.asarray(inputs["fp_b3"]).reshape(-1)[0]) * NP
    )
    return W1P.astype(np.float16), W2P, B2P, W3P, np.array([[b3sum]], np.float32)


def make_in_maps(inputs, bpc=BPC, ncores=NCORES):
    W1P, W2P, B2P, W3P, B3S = pack_weights(inputs)
    WALL = np.zeros((H, 6 * H), np.float16)
    for j in range(3):
        WALL[0:KROWS, H * j: H * (j + 1)] = W1P[j]
        WALL[:, H * (3 + j): H * (4 + j)] = W2P[j]
    BW = np.zeros((H, 8), np.float32)
    for j in range(3):
        BW[:, j] = B2P[j][:, 0]
        BW[:, 3 + j] = W3P[j][:, 0]
    BW[0, 6] = B3S[0, 0]
    emb16 = np.asarray(inputs["emb"], np.float32).astype(np.float16)
    seq = np.asarray(inputs["seq"], np.int64)
    R = np.asarray(inputs["R"], np.float32)
    e_all = emb16[seq]                       # [B, L, E]
    consts = dict(S48=_sel48(), WALL=WALL, BW=BW)
    in_maps = []
    for c in range(ncores):
        sl = slice(c * bpc, (c + 1) * bpc)
        Rt = np.ascontiguousarray(R[sl].transpose(2, 0, 1))        # [3, bpc, L]
        e = e_all[sl]                                              # [bpc, L, E]
        SE = np.zeros((bpc, 65, L), np.float16)
        for j in range(4):
            SE[:, 16 * j: 16 * j + 16, : L - j] = e[:, j:, :].transpose(0, 2, 1)
        SE[:, 64, :] = 1.0
        m = dict(consts)
        m["Rt"] = Rt
        m["SE"] = SE
        in_maps.append(m)
    return in_maps


def kernel(**inputs):
    nc = get_nc()
    in_maps = make_in_maps(inputs)
    res = bass_utils.run_bass_kernel_spmd(nc, in_maps, core_ids=list(range(NCORES)))
    return np.concatenate([res.results[c]["out"][0] for c in range(NCORES)]).astype(np.float32)


# revision 16
# speedup vs baseline: 2.7300x; 1.0101x over previous
"""Trainium2 Bass kernel for nn_LocalEnergy (protein local-energy GNN).

kernel(**inputs) takes FULL unsharded inputs (B=128), shards B across 8
NeuronCores (16 samples/core, pure data parallel), runs one Bass kernel
SPMD, gathers per-core [16] energies into the full [128] output.

v2 layout:
 - Host prep (indexing/layout only): embedding gather emb[seq] replicated
   into 4 shifted row-blocks + ones row -> SE [16, 65, 2048] fp16 per core;
   R transposed to [3, 16, L]; W1 packed (zero-padded, bias folded via the
   ones row, torsion sin-row sign-flipped) to match the on-device stack
   row layout.
 - Device phase 1 (geometry): fp16 vector math, rotations/shifts via DMA
   (no gpsimd), fp16 selection matmuls, Ln/Exp on scalar engine. Produces
   feature tile F [64, L] fp16 = [sin | cos | len-3.8 | cos_theta] blocks.
 - Device phase 2 (MLPs): per sample, stack [69, L] fp16 = SE rows + F
   rows; 3x fused W1 matmuls (K=69) per 512-chunk, relu on vector engine
   per 1024-col half, W2 matmuls, and one scalar-engine Relu+accumulate
   over all 2047-ish columns per (sample, MLP).
"""

import sys
import types
import numpy as np
from contextlib import ExitStack


def ensure_axon_hooks():
    """The container's antenv is a stub without axon_hooks; inject it so
    run_bass_kernel_spmd(trace=True) can NTFF-profile."""
    if "antenv.axon_hooks" in sys.modules:
        return
    import antenv

    hooks = types.ModuleType("antenv.axon_hooks")
    hooks._h = None

    def set_axon_ntff_profile_hook(h):
        hooks._h = h

    def get_axon_ntff_profile_hook():
        return hooks._h

    hooks.set_axon_ntff_profile_hook = set_axon_ntff_profile_hook
    hooks.get_axon_ntff_profile_hook = get_axon_ntff_profile_hook
    sys.modules["antenv.axon_hooks"] = hooks
    antenv.axon_hooks = hooks
    try:
        from trn_agent_boot.trn_boot import _ntff_profile_via_ctypes

        hook = _ntff_profile_via_ctypes("/opt/axon/libaxon_pjrt.so")
        if hook is not None:
            set_axon_ntff_profile_hook(hook)
    except Exception:
        pass


ensure_axon_hooks()

import concourse.bass as bass  # noqa: E402
import concourse.tile as tile  # noqa: E402
from concourse import mybir, bacc, bass_utils  # noqa: E402

dt = mybir.dt
AF = mybir.ActivationFunctionType
ALU = mybir.AluOpType
AX = mybir.AxisListType

NCORES = 8
B, L, NAA, E, H = 128, 2048, 20, 16, 128
BPC = B // NCORES
W = 512
NCH = L // W                       # 4 chunks of 512 per sample
NL, NT, NP = L - 1, L - 2, L - 3
KROWS = 69                         # stack rows: 64 emb-shift + ones + 4 features
SINV = 1.0 / 16.0                  # cross-product scaling to stay in fp16 range

MLPS = ("fl", "ft", "fp")
W1K = {"fl": 66, "ft": 67, "fp": 69}   # stack rows each MLP reads
KOFF = {"fl": 1, "ft": 2, "fp": 3}   # valid cols per sample = L - KOFF


def build_nc(bpc=BPC, ll=L):
    nc = bacc.Bacc("TRN2", target_bir_lowering=False, debug=False)

    Rt_d = nc.dram_tensor("Rt", (3, bpc, ll), dt.float32, kind="ExternalInput")
    SE_d = nc.dram_tensor("SE", (bpc, 65, ll), dt.float16, kind="ExternalInput")
    S48_d = nc.dram_tensor("S48", (48, 16), dt.float16, kind="ExternalInput")
    WW_d = nc.dram_tensor("WALL", (H, 6 * H), dt.float16, kind="ExternalInput")
    BW_d = nc.dram_tensor("BW", (H, 8), dt.float32, kind="ExternalInput")
    out_d = nc.dram_tensor("out", (1, bpc), dt.float32, kind="ExternalOutput")

    nl, nt, np_ = ll - 1, ll - 2, ll - 3

    with tile.TileContext(nc) as tc, ExitStack() as ctx:
        consts = ctx.enter_context(tc.tile_pool(name="consts", bufs=1))
        # rt first: phase 1 is gated on it, so its DMAs lead the sync queue
        rt = consts.tile([48, ll], dt.float32, name="rt")
        for c in range(3):
            nc.sync.dma_start(out=rt[16 * c: 16 * c + bpc, :], in_=Rt_d.ap()[c])
        s48 = consts.tile([48, 16], dt.float16)
        nc.sync.dma_start(out=s48, in_=S48_d.ap())
        wall = consts.tile([H, 6 * H], dt.float16, name="wall")
        nc.sync.dma_start(out=wall, in_=WW_d.ap())
        bw = consts.tile([H, 8], dt.float32, name="bw")
        nc.sync.dma_start(out=bw, in_=BW_d.ap())
        w1, w2, b2c, w3c = {}, {}, {}, {}
        for j, m in enumerate(MLPS):
            w1[m] = wall[0:W1K[m], H * j: H * (j + 1)]
            w2[m] = wall[:, H * (3 + j): H * (4 + j)]
            b2c[m] = bw[:, j: j + 1]
            w3c[m] = bw[:, 3 + j: 4 + j]
        b3s = bw[0:1, 6:7]

        # feature tile: rows 32f+s, f = 0:sin' 1:cos(phi) 2:len-3.8 3:cos(theta)
        F = consts.tile([128, ll], dt.float16, name="F")
        nc.vector.memset(F, 0.0)
        eps_t = consts.tile([16, 1], dt.float32, name="eps_t")
        nc.vector.memset(eps_t, 1e-6)
        lnsinv_t = consts.tile([16, 1], dt.float32, name="lnsinv_t")
        nc.vector.memset(lnsinv_t, float(np.log(SINV)))

        acc = {}
        for m in MLPS:
            acc[m] = consts.tile([H, 2 * bpc], dt.float32, name=f"acc_{m}")

        # stack pool opens before phase 1 so SE loads prefetch under geometry
        stk = ctx.enter_context(tc.tile_pool(name="stk", bufs=4))
        stack_tiles = {}

        def alloc_stack(s):
            t = stk.tile([KROWS, ll], dt.float16, name="stack", tag="stk")
            nc.sync.dma_start(out=t[0:65, :], in_=SE_d.ap()[s])
            stack_tiles[s] = t

        for s in range(4):
            alloc_stack(s)

        # ---------------- Phase 1: geometry ----------------
        with tc.tile_pool(name="geo", bufs=1) as geo, \
             tc.tile_pool(name="geo_ps", bufs=2, space="PSUM") as geo_ps:
            def g16(name, cols):
                return geo.tile([48, cols], dt.float16, name=name)

            A1 = g16("A1", nt)
            A2 = g16("A2", nt)
            B1 = g16("B1", nt)
            B2 = g16("B2", nt)
            C1s = g16("C1s", np_)

            D = g16("D", nl)
            nc.vector.tensor_tensor(out=D, in0=rt[:, 1:ll], in1=rt[:, 0:nl], op=ALU.subtract)
            D1 = g16("D1", nt)
            nc.vector.tensor_tensor(out=D1, in0=rt[:, 2:ll], in1=rt[:, 1:nl], op=ALU.subtract)
            Ds = g16("Ds", nl)
            nc.vector.tensor_scalar(out=Ds, in0=D, scalar1=SINV, scalar2=None, op0=ALU.mult)
            D1s = g16("D1s", nt)
            nc.vector.tensor_scalar(out=D1s, in0=D1, scalar1=SINV, scalar2=None, op0=ALU.mult)

            DSQ = g16("DSQ", nl)
            nc.vector.tensor_tensor(out=DSQ, in0=D, in1=D, op=ALU.mult)
            DD = g16("DD", nt)
            nc.vector.tensor_tensor(out=DD, in0=D[:, 0:nt], in1=D1, op=ALU.mult)

            # coordinate rotations of the scaled bond vectors, via DMA
            for c in range(3):
                c1, c2 = (c + 1) % 3, (c + 2) % 3
                nc.sync.dma_start(out=A1[16 * c: 16 * c + bpc, :], in_=Ds[16 * c1: 16 * c1 + bpc, 0:nt])
                nc.sync.dma_start(out=A2[16 * c: 16 * c + bpc, :], in_=Ds[16 * c2: 16 * c2 + bpc, 0:nt])
                # B rotations issue from the scalar-engine HWDGE ring in parallel
                nc.scalar.dma_start(out=B1[16 * c: 16 * c + bpc, :], in_=D1s[16 * c1: 16 * c1 + bpc, :])
                nc.scalar.dma_start(out=B2[16 * c: 16 * c + bpc, :], in_=D1s[16 * c2: 16 * c2 + bpc, :])

            t_a = g16("t_a", nt)
            nc.vector.tensor_tensor(out=t_a, in0=A1, in1=B2, op=ALU.mult)
            t_b = g16("t_b", nt)
            nc.vector.tensor_tensor(out=t_b, in0=A2, in1=B1, op=ALU.mult)
            Cs = g16("Cs", nt)
            nc.vector.tensor_tensor(out=Cs, in0=t_a, in1=t_b, op=ALU.subtract)
            for c in range(3):
                nc.sync.dma_start(out=C1s[16 * c: 16 * c + bpc, :], in_=Cs[16 * c: 16 * c + bpc, 1: 1 + np_])

            XR = g16("XR", np_)
            nc.vector.tensor_tensor(out=XR, in0=Cs[:, 0:np_], in1=C1s, op=ALU.mult)
            YR = g16("YR", np_)
            nc.vector.tensor_tensor(out=YR, in0=Ds[:, 0:np_], in1=C1s, op=ALU.mult)

            def selmm(dst, src, count):
                for c0 in range(0, count, W):
                    n = min(W, count - c0)
                    nc.tensor.matmul(dst[:, c0 // W, :n], s48, src[:, c0: c0 + n], start=True, stop=True)

            lsq_ps = geo_ps.tile([16, NCH, W], dt.float32, name="lsq_ps", tag="gps")
            selmm(lsq_ps, DSQ, nl)
            lnl = geo.tile([16, nl], dt.float16, name="lnl")
            nc.scalar.activation(out=lnl, in_=lsq_ps.rearrange("p a b -> p (a b)")[:, 0:nl], func=AF.Ln)
            rlen = geo.tile([16, nl], dt.float16, name="rlen")
            nc.scalar.activation(out=rlen, in_=lnl, func=AF.Exp, scale=-0.5)
            lenf = geo.tile([16, nl], dt.float16, name="lenf")
            nc.scalar.activation(out=lenf, in_=lnl, func=AF.Exp, scale=0.5)
            lenf16 = geo.tile([16, nl], dt.float16, name="lenf16")
            nc.scalar.activation(out=lenf16, in_=lnl, func=AF.Exp, scale=0.5, bias=lnsinv_t)
            # len feature: |d| - 3.8 (b1_fl adjusted host-side)
            nc.vector.tensor_scalar(out=F[0:16, 0:nl], in0=lenf, scalar1=3.8, scalar2=None, op0=ALU.subtract)

            rlen1 = geo.tile([16, nt], dt.float16, name="rlen1")
            nc.scalar.dma_start(out=rlen1, in_=rlen[:, 1: 1 + nt])
            lenf1 = geo.tile([16, np_], dt.float16, name="lenf1")
            nc.scalar.dma_start(out=lenf1, in_=lenf16[:, 1: 1 + np_])

            dot_ps = geo_ps.tile([16, NCH, W], dt.float32, name="dot_ps", tag="gps")
            selmm(dot_ps, DD, nt)
            tt1 = geo.tile([16, nt], dt.float16, name="tt1")
            nc.vector.tensor_tensor(out=tt1, in0=dot_ps.rearrange("p a b -> p (a b)")[:, 0:nt], in1=rlen[:, 0:nt], op=ALU.mult)
            # store +(d.d')*rlen_i*rlen_{i+1} = -cos(theta); sign folded into W1_ft
            nc.vector.tensor_tensor(out=F[32:48, 0:nt], in0=tt1, in1=rlen1, op=ALU.mult)

            xr_ps = geo_ps.tile([16, NCH, W], dt.float32, name="xr_ps", tag="gps")
            selmm(xr_ps, XR, np_)
            x_sb = geo.tile([16, np_], dt.float16, name="x_sb")
            nc.scalar.activation(out=x_sb, in_=xr_ps.rearrange("p a b -> p (a b)")[:, 0:np_], func=AF.Copy)
            yr_ps = geo_ps.tile([16, NCH, W], dt.float32, name="yr_ps", tag="gps")
            selmm(yr_ps, YR, np_)
            y_sb = geo.tile([16, np_], dt.float16, name="y_sb")
            # y = (b1 . n2) * |b2| / 16 (the 1/16 rides in lenf16) -> matches x's s^4 scale
            nc.vector.tensor_tensor(
                out=y_sb, in0=yr_ps.rearrange("p a b -> p (a b)")[:, 0:np_],
                in1=lenf1, op=ALU.mult)

            q1 = geo.tile([16, np_], dt.float16, name="q1")
            nc.vector.tensor_tensor(out=q1, in0=x_sb, in1=x_sb, op=ALU.mult)
            q2 = geo.tile([16, np_], dt.float16, name="q2")
            nc.vector.tensor_tensor(out=q2, in0=y_sb, in1=y_sb, op=ALU.mult)
            q = geo.tile([16, np_], dt.float16, name="q")
            nc.vector.tensor_tensor(out=q, in0=q1, in1=q2, op=ALU.add)
            lnq = geo.tile([16, np_], dt.float16, name="lnq")
            nc.scalar.activation(out=lnq, in_=q, func=AF.Ln, bias=eps_t)
            r2v = geo.tile([16, np_], dt.float16, name="r2v")
            nc.scalar.activation(out=r2v, in_=lnq, func=AF.Exp, scale=-0.5)
            # sin' = -sin(phi): sign folded into W1 sin rows host-side
            nc.vector.tensor_tensor(out=F[64:80, 0:np_], in0=y_sb, in1=r2v, op=ALU.mult)
            nc.vector.tensor_tensor(out=F[96:112, 0:np_], in0=x_sb, in1=r2v, op=ALU.mult)

        # ---------------- Phase 2: MLPs ----------------
        with tc.tile_pool(name="h1_ps", bufs=2, space="PSUM") as h1_ps, \
             tc.tile_pool(name="h1r_p", bufs=13) as h1r_p, \
             tc.tile_pool(name="h2_ps", bufs=2, space="PSUM") as h2_ps, \
             tc.tile_pool(name="scr_p", bufs=2) as scr_p:

            h1r_ref = {}

            def emit_w1(s, m):
                stack = stack_tiles[s]
                for h in range(2):
                    h1 = h1_ps.tile([H, 2, W], dt.float32, name="h1", tag="h1ps")
                    for ci in range(2):
                        c0 = (2 * h + ci) * W
                        n = min(W, (ll - KOFF[m]) - c0)
                        nc.tensor.matmul(h1[:, ci, :n], w1[m], stack[0:W1K[m], c0: c0 + n], start=True, stop=True)
                    h1r = h1r_p.tile([H, 2, W], dt.float16, name="h1r", tag="h1r")
                    nc.vector.tensor_scalar(
                        out=h1r.rearrange("p a b -> p (a b)"),
                        in0=h1.rearrange("p a b -> p (a b)"),
                        scalar1=0.0, scalar2=None, op0=ALU.max)
                    h1r_ref[(s, m, h)] = h1r

            def emit_w2(s, m):
                nv = ll - KOFF[m]
                for hh in range(2):
                    h2 = h2_ps.tile([H, 2, W], dt.float32, name="h2", tag="h2ps")
                    for ci in range(2):
                        c = 2 * hh + ci
                        n = min(W, nv - c * W)
                        h1r = h1r_ref[(s, m, hh)]
                        nc.tensor.matmul(h2[:, ci, :n], w2[m], h1r[:, ci, :n], start=True, stop=True)
                    nh = min(2 * W, nv - hh * 2 * W)
                    scr = scr_p.tile([H, 2, W], dt.float16, name="scr", tag="scr")
                    nc.scalar.activation(
                        out=scr.rearrange("p a b -> p (a b)")[:, 0:nh],
                        in_=h2.rearrange("p a b -> p (a b)")[:, 0:nh],
                        func=AF.Relu, bias=b2c[m],
                        accum_out=acc[m][:, 2 * s + hh: 2 * s + hh + 1])

            def prep_sample(s):
                if s not in stack_tiles:
                    alloc_stack(s)
                if s + 2 < bpc and (s + 2) not in stack_tiles:
                    alloc_stack(s + 2)     # keep the SE prefetch 2 samples ahead
                stack = stack_tiles[s]
                Fv = F.rearrange("(f s) l -> f s l", s=32)
                nc.sync.dma_start(out=stack[65:67, :], in_=Fv[0:2, s, :])
                nc.sync.dma_start(out=stack[67:69, :], in_=Fv[2:4, s, :])

            for s in range(bpc + 1):
                if s < bpc:
                    prep_sample(s)
                for m in MLPS:
                    if s < bpc:
                        emit_w1(s, m)
                    if s >= 1:
                        emit_w2(s - 1, m)

        # ---------------- final reduction ----------------
        with tc.tile_pool(name="fin_ps", bufs=1, space="PSUM") as fin_ps:
            ep = fin_ps.tile([1, 3, 2 * bpc], dt.float32, name="ep")
            for j, m in enumerate(MLPS):
                nc.tensor.matmul(ep[:, j, :], w3c[m], acc[m], start=True, stop=True)
            esum = consts.tile([1, bpc], dt.float32, name="esum")
            nc.vector.tensor_reduce(
                out=esum, in_=ep.rearrange("o m (s h) -> o s m h", h=2), axis=AX.XY, op=ALU.add)
            eout = consts.tile([1, bpc], dt.float32, name="eout")
            nc.vector.tensor_scalar(out=eout, in0=esum, scalar1=b3s, scalar2=None, op0=ALU.add)
            nc.sync.dma_start(out=out_d.ap(), in_=eout)

    nc.finalize()
    return nc


_NC_CACHE = {}


def get_nc(bpc=BPC, ll=L):
    key = (bpc, ll)
    if key not in _NC_CACHE:
        _NC_CACHE[key] = build_nc(bpc, ll)
    return _NC_CACHE[key]


def _sel48():
    S = np.zeros((48, 16), np.float16)
    for c in range(3):
        for s in range(16):
            S[16 * c + s, s] = 1.0
    return S


def pack_weights(inputs):
    """Pack per-MLP W1 into the [69, H] stack-row layout (fp16), fold b1 via
    the ones row, flip the torsion sin-row sign, fold the 3.8 len-centering
    into b1_fl."""
    f32 = lambda k: np.asarray(inputs[k], np.float32)
    W1P = np.zeros((3, KROWS, H), np.float32)
    # fl: x = [len, e0, e1]
    w = f32("fl_W1")
    W1P[0, 0:32] = w[1:33]
    W1P[0, 65] = w[0]
    W1P[0, 64] = f32("fl_b1") + 3.8 * w[0]
    # ft: x = [cos_t, e0, e1, e2]
    w = f32("ft_W1")
    W1P[1, 0:48] = w[1:49]
    W1P[1, 66] = -w[0]          # device stores -cos(theta)
    W1P[1, 64] = f32("ft_b1")
    # fp: x = [sin, cos, e0, e1, e2, e3]
    w = f32("fp_W1")
    W1P[2, 0:64] = w[2:66]
    W1P[2, 67] = -w[0]          # device computes -sin
    W1P[2, 68] = w[1]
    W1P[2, 64] = f32("fp_b1")
    W2P = np.stack([f32(f"{m}_W2") for m in MLPS]).astype(np.float16)
    B2P = np.stack([f32(f"{m}_b2").reshape(H, 1) for m in MLPS]).astype(np.float32)
    W3P = np.stack([f32(f"{m}_W3") for m in MLPS]).astype(np.float32)
    b3sum = np.float32(
        float(np.asarray(inputs["fl_b3"]).reshape(-1)[0]) * NL
        + float(np.asarray(inputs["ft_b3"]).reshape(-1)[0]) * NT
        + float(np.asarray(inputs["fp_b3"]).reshape(-1)[0]) * NP
    )
    return W1P.astype(np.float16), W2P, B2P, W3P, np.array([[b3sum]], np.float32)


def make_in_maps(inputs, bpc=BPC, ncores=NCORES):
    W1P, W2P, B2P, W3P, B3S = pack_weights(inputs)
    WALL = np.zeros((H, 6 * H), np.float16)
    for j in range(3):
        WALL[0:KROWS, H * j: H * (j + 1)] = W1P[j]
        WALL[:, H * (3 + j): H * (4 + j)] = W2P[j]
    BW = np.zeros((H, 8), np.float32)
    for j in range(3):
        BW[:, j] = B2P[j][:, 0]
        BW[:, 3 + j] = W3P[j][:, 0]
    BW[0, 6] = B3S[0, 0]
    emb16 = np.asarray(inputs["emb"], np.float32).astype(np.float16)
    seq = np.asarray(inputs["seq"], np.int64)
    R = np.asarray(inputs["R"], np.float32)
    e_all = emb16[seq]                       # [B, L, E]
    consts = dict(S48=_sel48(), WALL=WALL, BW=BW)
    in_maps = []
    for c in range(ncores):
        sl = slice(c * bpc, (c + 1) * bpc)
        Rt = np.ascontiguousarray(R[sl].transpose(2, 0, 1))        # [3, bpc, L]
        e = e_all[sl]                                              # [bpc, L, E]
        SE = np.zeros((bpc, 65, L), np.float16)
        for j in range(4):
            SE[:, 16 * j: 16 * j + 16, : L - j] = e[:, j:, :].transpose(0, 2, 1)
        SE[:, 64, :] = 1.0
        m = dict(consts)
        m["Rt"] = Rt
        m["SE"] = SE
        in_maps.append(m)
    return in_maps


def kernel(**inputs):
    nc = get_nc()
    in_maps = make_in_maps(inputs)
    res = bass_utils.run_bass_kernel_spmd(nc, in_maps, core_ids=list(range(NCORES)))
    return np.concatenate([res.results[c]["out"][0] for c in range(NCORES)]).astype(np.float32)
